# revision 4
# baseline (speedup 1.0000x reference)
"""AdaptivePeriodicLayer Trainium2 kernel.

Strategy: data-parallel over batch (8 samples -> 8 NeuronCores), one SPMD
program with an 8-way tc.Switch on partition id. Each arm is specialized at
build time to that sample's 3 FFT-detected period geometries (computed on
host, like the reference's .item() round-trip). Convs are computed as
per-kernel-offset matmuls accumulated in PSUM over a zero-padded 2D canvas
held in SBUF; conv operands are bf16 (fp32 PSUM accumulation), everything
else fp32.
"""

import numpy as np

import concourse.bacc as bacc
import concourse.bass as bass
import concourse.mybir as mybir
from concourse.tile import TileContext
from concourse.bass_utils import run_bass_kernel_spmd

F32 = mybir.dt.float32
BF16 = mybir.dt.bfloat16
AF = mybir.ActivationFunctionType
AX = mybir.AxisListType

B, S, D, DFF, K = 8, 512, 128, 256, 3
NOFF = 35  # 1 + 9 + 25 conv taps across the three inception branches
WCHUNK = 7  # offsets per weight DMA chunk
NWCH = 5    # 35 / 7

_last_exec_time_ns = None
_last_results = None


def _offsets():
    offs = []
    for ksz in (1, 3, 5):
        q = (ksz - 1) // 2
        for kh in range(ksz):
            for kw in range(ksz):
                offs.append((q, kh, kw))
    return offs


OFFS = _offsets()


def _row_chunks(p, n):
    """Split the p x n image (h-major) into row groups of <=512 columns."""
    ch = max(1, min(p, 512 // n))
    out = []
    h0 = 0
    while h0 < p:
        rows = min(ch, p - h0)
        out.append((h0, rows))
        h0 += rows
    return out


def _build_program(geoms):
    """geoms: [8][3] list of (p, n) per core per pass."""
    nc = bacc.Bacc()

    xb = nc.declare_dram_parameter("xb", [S, D], F32, isOutput=False)
    w1 = nc.declare_dram_parameter("w1", [D, NOFF * 2 * 128], BF16, isOutput=False)
    w2 = nc.declare_dram_parameter("w2", [D, NOFF * 2 * 128], BF16, isOutput=False)
    b1 = nc.declare_dram_parameter("b1", [DFF, 1], F32, isOutput=False)
    eb = nc.declare_dram_parameter("eb", [D, 1], F32, isOutput=False)
    enh = nc.declare_dram_parameter("enh", [D, D], F32, isOutput=False)
    ident = nc.declare_dram_parameter("ident", [128, 128], F32, isOutput=False)
    aw1 = nc.declare_dram_parameter("aw1", [1, K * 2 * K], F32, isOutput=False)
    aw2 = nc.declare_dram_parameter("aw2", [1, 2 * K * K], F32, isOutput=False)
    ab1 = nc.declare_dram_parameter("ab1", [1, 2 * K], F32, isOutput=False)
    ab2 = nc.declare_dram_parameter("ab2", [1, K], F32, isOutput=False)
    pwd = nc.declare_dram_parameter("pw", [1, K], F32, isOutput=False)
    out_d = nc.declare_dram_parameter("out", [S, D], F32, isOutput=True)

    with TileContext(nc) as tc:
        with (
            tc.tile_pool(name="persist", bufs=1) as pp,
            tc.tile_pool(name="work", bufs=1) as wp,
            tc.tile_pool(name="psum", bufs=1, space="PSUM") as qp,
        ):
            # ---- persistent loads (identical data on every core) ----
            identS = pp.tile([128, 128], F32, tag="identS", name="identS")
            nc.sync.dma_start(out=identS, in_=ident[:, :])

            xall = pp.tile([128, S], F32, tag="xall", name="xall")
            for c in range(4):
                nc.sync.dma_start(out=xall[:, c * 128:(c + 1) * 128],
                                  in_=xb[c * 128:(c + 1) * 128, :])

            w1c = []
            for ci in range(NWCH):
                t = pp.tile([128, WCHUNK * 2 * 128], BF16, tag=f"w1c{ci}",
                            name=f"w1c{ci}")
                nc.sync.dma_start(
                    out=t, in_=w1[:, ci * WCHUNK * 256:(ci + 1) * WCHUNK * 256])
                w1c.append(t)

            b1S = pp.tile([128, 2], F32, tag="b1S", name="b1S")
            nc.sync.dma_start(out=b1S[:, 0:1], in_=b1[0:128, :])
            nc.sync.dma_start(out=b1S[:, 1:2], in_=b1[128:256, :])
            ebS = pp.tile([128, 1], F32, tag="ebS", name="ebS")
            nc.sync.dma_start(out=ebS, in_=eb[:, :])
            enhS = pp.tile([128, 128], F32, tag="enhS", name="enhS")
            nc.sync.dma_start(out=enhS, in_=enh[:, :])
            aw1S = pp.tile([1, K * 2 * K], F32, tag="aw1S", name="aw1S")
            nc.sync.dma_start(out=aw1S, in_=aw1[:, :])
            aw2S = pp.tile([1, 2 * K * K], F32, tag="aw2S", name="aw2S")
            nc.sync.dma_start(out=aw2S, in_=aw2[:, :])
            ab1S = pp.tile([1, 2 * K], F32, tag="ab1S", name="ab1S")
            nc.sync.dma_start(out=ab1S, in_=ab1[:, :])
            ab2S = pp.tile([1, K], F32, tag="ab2S", name="ab2S")
            nc.sync.dma_start(out=ab2S, in_=ab2[:, :])
            pwS = pp.tile([1, K], F32, tag="pwS", name="pwS")
            nc.sync.dma_start(out=pwS, in_=pwd[:, :])

            w2c = []
            for ci in range(NWCH):
                t = pp.tile([128, WCHUNK * 2 * 128], BF16, tag=f"w2c{ci}",
                            name=f"w2c{ci}")
                nc.sync.dma_start(
                    out=t, in_=w2[:, ci * WCHUNK * 256:(ci + 1) * WCHUNK * 256])
                w2c.append(t)

            # constants
            onesM = pp.tile([128, 1], F32, tag="onesM", name="onesM")
            nc.vector.memset(onesM, 1.0 / 128.0)
            ones1_128 = pp.tile([1, 128], F32, tag="ones1_128", name="ones1_128")
            nc.vector.memset(ones1_128, 1.0)

            # ---- transpose x: [S, D] -> xT [D, S] ----
            xT = pp.tile([128, S], F32, tag="xT", name="xT")
            for c in range(4):
                pt = qp.tile([128, 512], F32, tag="ps_small", bufs=2,
                             name=f"ptx{c}")
                nc.tensor.transpose(pt[:, 0:128],
                                    xall[:, c * 128:(c + 1) * 128], identS)
                nc.vector.tensor_copy(out=xT[:, c * 128:(c + 1) * 128],
                                      in_=pt[:, 0:128])

            def w1_lhsT(o, mc):
                ci, oi = divmod(o, WCHUNK)
                base = (oi * 2 + mc) * 128
                return w1c[ci][:, base:base + 128]

            def w2_lhsT(o, kc):
                ci, oi = divmod(o, WCHUNK)
                base = (oi * 2 + kc) * 128
                return w2c[ci][:, base:base + 128]

            def build_arm(core):
                # allocate per-pass tiles; memset canvases early so DVE works
                # while the weight DMAs stream in
                cvs, hcs = [], []
                for k in range(K):
                    p, n = geoms[core][k]
                    Hp, Wp = p + 4, n + 4
                    canvas = wp.tile([128, Hp * Wp], BF16, tag=f"canvas{k}",
                                     name=f"canvas{k}_{core}")
                    nc.vector.memset(canvas, 0.0)
                    cvs.append(canvas.rearrange("d (h w) -> d h w", w=Wp))
                    hk = []
                    for mc in range(2):
                        hc = wp.tile([128, Hp * Wp], BF16, tag=f"hc{k}_{mc}",
                                     name=f"hc{k}_{mc}_{core}")
                        nc.vector.memset(hc, 0.0)
                        hk.append(hc.rearrange("d (h w) -> d h w", w=Wp))
                    hcs.append(hk)

                po, poT = [], []
                polastT = wp.tile([128, K], F32, tag="polast",
                                  name=f"polast_{core}")
                for k in range(K):
                    p, n = geoms[core][k]
                    L = p * n
                    nfull = S // p
                    r = S - nfull * p
                    chunks = _row_chunks(p, n)
                    assert len(chunks) <= 2, (p, n, chunks)
                    cv, hcv = cvs[k], hcs[k]

                    src = xT[:, 0:nfull * p].rearrange("d (w h) -> d h w", h=p)
                    nc.vector.tensor_copy(out=cv[:, 2:2 + p, 2:2 + nfull],
                                          in_=src)
                    if r > 0:
                        tsrc = xT[:, nfull * p:S].rearrange(
                            "d (h w) -> d h w", w=1)
                        nc.vector.tensor_copy(
                            out=cv[:, 2:2 + r, 2 + nfull:3 + nfull], in_=tsrc)

                    # conv1: accumulate 35 taps per (Cout chunk, row chunk)
                    ps1 = [qp.tile([128, 1024], F32, tag=f"c1p{mc}",
                                   name=f"c1p{mc}_{core}_{k}")
                           for mc in range(2)]
                    for mc in range(2):
                        for o, (q, kh, kw) in enumerate(OFFS):
                            for j, (h0, rows) in enumerate(chunks):
                                rhs = cv[:, 2 - q + kh + h0:
                                         2 - q + kh + h0 + rows,
                                         2 - q + kw:2 - q + kw + n]
                                nc.tensor.matmul(
                                    ps1[mc][:, j * 512:j * 512 + rows * n],
                                    w1_lhsT(o, mc), rhs,
                                    start=(o == 0), stop=(o == NOFF - 1))
                        # gelu(conv1 + b1) written into padded h-canvas
                        for j, (h0, rows) in enumerate(chunks):
                            nc.scalar.activation(
                                out=hcv[mc][:, 2 + h0:2 + h0 + rows, 2:2 + n],
                                in_=ps1[mc][:, j * 512:j * 512 + rows * n],
                                func=AF.Gelu, bias=b1S[:, mc:mc + 1])

                    # conv2: contract over 256 channels (2 K-chunks) x 35 taps
                    ps2 = qp.tile([128, 1024], F32, tag="c2p",
                                  name=f"c2p_{core}_{k}")
                    for kc in range(2):
                        for o, (q, kh, kw) in enumerate(OFFS):
                            for j, (h0, rows) in enumerate(chunks):
                                rhs = hcv[kc][:, 2 - q + kh + h0:
                                              2 - q + kh + h0 + rows,
                                              2 - q + kw:2 - q + kw + n]
                                nc.tensor.matmul(
                                    ps2[:, j * 512:j * 512 + rows * n],
                                    w2_lhsT(o, kc), rhs,
                                    start=(kc == 0 and o == 0),
                                    stop=(kc == 1 and o == NOFF - 1))

                    # conv2 out back to SBUF, contiguous h-major
                    c2 = wp.tile([128, 1024], F32, tag="c2",
                                 name=f"c2_{core}_{k}")
                    for j, (h0, rows) in enumerate(chunks):
                        nc.vector.tensor_copy(
                            out=c2[:, h0 * n:h0 * n + rows * n],
                            in_=ps2[:, j * 512:j * 512 + rows * n])

                    # period enhancement on the first S sequence positions
                    ps3 = qp.tile([128, 512], F32, tag="ps_small", bufs=2,
                                  name=f"ps3_{core}_{k}")
                    c2v = c2[:, 0:L].rearrange("d (h w) -> d w h", w=n)
                    nc.tensor.matmul(ps3[:, 0:nfull * p], enhS,
                                     c2v[:, 0:nfull, :],
                                     start=True, stop=(r == 0))
                    if r > 0:
                        nc.tensor.matmul(ps3[:, nfull * p:S], enhS,
                                         c2v[:, nfull:nfull + 1, 0:r],
                                         start=False, stop=True)
                    po_k = wp.tile([128, S], F32, tag=f"po{k}",
                                   name=f"po{k}_{core}")
                    nc.scalar.activation(out=po_k, in_=ps3[:, 0:S],
                                         func=AF.Gelu, bias=ebS[:, 0:1])
                    po.append(po_k)
                    nc.vector.tensor_copy(out=polastT[:, k:k + 1],
                                          in_=po_k[:, S - 1:S])
                    # eager transpose to [s, d] chunks (overlaps next pass)
                    poT_k = wp.tile([128, S], F32, tag=f"poT{k}",
                                    name=f"poT{k}_{core}")
                    for c in range(4):
                        pt = qp.tile([128, 512], F32, tag="ps_small", bufs=2,
                                     name=f"ptp{k}{c}_{core}")
                        nc.tensor.transpose(pt[:, 0:128],
                                            po_k[:, c * 128:(c + 1) * 128],
                                            identS)
                        nc.vector.tensor_copy(
                            out=poT_k[:, c * 128:(c + 1) * 128],
                            in_=pt[:, 0:128])
                    poT.append(poT_k)

                # ---- combination weights (free-dim chain on partition 0) ----
                lsfP = qp.tile([1, K], F32, tag="ps_small", bufs=2,
                               name=f"lsfP_{core}")
                nc.tensor.matmul(lsfP, onesM, polastT, start=True, stop=True)
                lsfS = wp.tile([1, K], F32, tag="lsfS", name=f"lsfS_{core}")
                nc.vector.tensor_copy(out=lsfS, in_=lsfP)

                t6 = wp.tile([1, 2 * K], F32, tag="t6", name=f"t6_{core}")
                tt6 = wp.tile([1, 2 * K], F32, tag="tt6", name=f"tt6_{core}")
                nc.vector.tensor_scalar_mul(t6, aw1S[:, 0:6], lsfS[:, 0:1])
                nc.vector.tensor_scalar_mul(tt6, aw1S[:, 6:12], lsfS[:, 1:2])
                nc.vector.tensor_add(t6, t6, tt6)
                nc.vector.tensor_scalar_mul(tt6, aw1S[:, 12:18], lsfS[:, 2:3])
                nc.vector.tensor_add(t6, t6, tt6)
                nc.vector.tensor_add(t6, t6, ab1S)
                nc.vector.tensor_scalar_max(t6, t6, 0.0)

                a3 = wp.tile([1, K], F32, tag="a3", name=f"a3_{core}")
                tt3 = wp.tile([1, K], F32, tag="tt3", name=f"tt3_{core}")
                nc.vector.tensor_scalar_mul(a3, aw2S[:, 0:3], t6[:, 0:1])
                for i in range(1, 6):
                    nc.vector.tensor_scalar_mul(tt3, aw2S[:, 3 * i:3 * i + 3],
                                                t6[:, i:i + 1])
                    nc.vector.tensor_add(a3, a3, tt3)
                nc.vector.tensor_add(a3, a3, ab2S)

                e1 = wp.tile([1, K], F32, tag="e1", name=f"e1_{core}")
                nc.scalar.activation(out=e1, in_=a3, func=AF.Exp)
                s1 = wp.tile([1, 1], F32, tag="s1", name=f"s1_{core}")
                nc.vector.reduce_sum(out=s1, in_=e1, axis=AX.X)
                r1 = wp.tile([1, 1], F32, tag="r1", name=f"r1_{core}")
                nc.vector.reciprocal(r1, s1)
                z = wp.tile([1, K], F32, tag="z", name=f"z_{core}")
                nc.vector.tensor_scalar_mul(z, e1, r1[:, 0:1])
                nc.vector.tensor_mul(z, z, pwS)
                e2 = wp.tile([1, K], F32, tag="e2", name=f"e2_{core}")
                nc.scalar.activation(out=e2, in_=z, func=AF.Exp)
                s2 = wp.tile([1, 1], F32, tag="s2", name=f"s2_{core}")
                nc.vector.reduce_sum(out=s2, in_=e2, axis=AX.X)
                r2 = wp.tile([1, 1], F32, tag="r2", name=f"r2_{core}")
                nc.vector.reciprocal(r2, s2)
                cwrow = wp.tile([1, K], F32, tag="cwrow", name=f"cwrow_{core}")
                nc.vector.tensor_scalar_mul(cwrow, e2, r2[:, 0:1])

                cwbP = qp.tile([128, K], F32, tag="ps_small", bufs=2,
                               name=f"cwbP_{core}")
                nc.tensor.matmul(cwbP, ones1_128, cwrow, start=True, stop=True)
                cwS = wp.tile([128, K], F32, tag="cwS", name=f"cwS_{core}")
                nc.vector.tensor_copy(out=cwS, in_=cwbP)

                # ---- weighted combine + residual, already in [s, d] ----
                accT = wp.tile([128, S], F32, tag="accT", name=f"accT_{core}")
                tmpT = wp.tile([128, S], F32, tag="tmpT", name=f"tmpT_{core}")
                nc.vector.tensor_scalar_mul(accT, poT[0], cwS[:, 0:1])
                nc.vector.tensor_scalar_mul(tmpT, poT[1], cwS[:, 1:2])
                nc.vector.tensor_add(accT, accT, tmpT)
                nc.vector.tensor_scalar_mul(tmpT, poT[2], cwS[:, 2:3])
                nc.vector.tensor_add(accT, accT, tmpT)
                nc.vector.tensor_add(accT, accT, xall)

                for c in range(4):
                    nc.sync.dma_start(out=out_d[c * 128:(c + 1) * 128, :],
                                      in_=accT[:, c * 128:(c + 1) * 128])

            pid = nc.partition_id(engines=mybir.ALL_ENGINES)
            for core in tc.Switch(pid, 8):
                build_arm(core)

    nc.finalize()
    return nc


def _prep_inputs(x, conv1_w, conv1_b, conv2_w, conv2_b, attn_w1, attn_b1,
                 attn_w2, attn_b2, enh_w, enh_b):
    x = np.asarray(x, dtype=np.float32)
    conv1_w = [np.asarray(w, dtype=np.float32) for w in conv1_w]
    conv1_b = [np.asarray(w, dtype=np.float32) for w in conv1_b]
    conv2_w = [np.asarray(w, dtype=np.float32) for w in conv2_w]
    conv2_b = [np.asarray(w, dtype=np.float32) for w in conv2_b]
    enh_w = np.asarray(enh_w, dtype=np.float32)
    enh_b = np.asarray(enh_b, dtype=np.float32)

    # FFT period detection on host (reference does the same via .item())
    xf = np.abs(np.fft.rfft(x, axis=1)).mean(-1).astype(np.float32)
    xf[:, 0] = 0.0
    top = np.argsort(-xf, axis=1, kind="stable")[:, :K]
    geoms, pws = [], []
    for b in range(B):
        g = []
        for k in range(K):
            f = int(top[b, k])
            p = S if f == 0 else max(1, S // f)
            pad = (-S) % p
            g.append((p, (S + pad) // p))
        geoms.append(g)
        wv = xf[b, top[b]].astype(np.float64)
        ev = np.exp(wv - wv.max())
        pws.append((ev / ev.sum()).astype(np.float32).reshape(1, K))

    # conv weights -> per-tap transposed matmul operands, pre-scaled by 1/3
    w1A = np.zeros((128, NOFF, 2, 128), dtype=np.float32)
    w2A = np.zeros((128, NOFF, 2, 128), dtype=np.float32)
    o = 0
    for br, ksz in enumerate((1, 3, 5)):
        for kh in range(ksz):
            for kw in range(ksz):
                wt1 = conv1_w[br][:, :, kh, kw] / 3.0   # [256(out), 128(in)]
                w1A[:, o, 0, :] = wt1[0:128, :].T
                w1A[:, o, 1, :] = wt1[128:256, :].T
                wt2 = conv2_w[br][:, :, kh, kw] / 3.0   # [128(out), 256(in)]
                w2A[:, o, 0, :] = wt2[:, 0:128].T
                w2A[:, o, 1, :] = wt2[:, 128:256].T
                o += 1
    import ml_dtypes
    w1A = w1A.reshape(128, NOFF * 2 * 128).astype(ml_dtypes.bfloat16)
    w2A = w2A.reshape(128, NOFF * 2 * 128).astype(ml_dtypes.bfloat16)
    b1m = (sum(conv1_b) / 3.0).reshape(DFF, 1).astype(np.float32)
    b2m = (sum(conv2_b) / 3.0).astype(np.float32)
    ebv = (b2m @ enh_w + enh_b).reshape(D, 1).astype(np.float32)

    shared = dict(
        w1=w1A, w2=w2A, b1=b1m, eb=ebv,
        enh=np.ascontiguousarray(enh_w),
        ident=np.eye(128, dtype=np.float32),
        aw1=np.asarray(attn_w1, dtype=np.float32).reshape(1, K * 2 * K),
        aw2=np.asarray(attn_w2, dtype=np.float32).reshape(1, 2 * K * K),
        ab1=np.asarray(attn_b1, dtype=np.float32).reshape(1, 2 * K),
        ab2=np.asarray(attn_b2, dtype=np.float32).reshape(1, K),
    )
    in_maps = []
    for b in range(B):
        m = dict(shared)
        m["xb"] = np.ascontiguousarray(x[b])
        m["pw"] = pws[b]
        in_maps.append(m)
    return geoms, in_maps


def _ensure_axon_hooks():
    """bass_utils' trace path imports antenv.axon_hooks unconditionally;
    the container's antenv lacks it. Provide it, registering the real
    ctypes NTFF hook when available so tracing works."""
    try:
        import antenv.axon_hooks  # noqa: F401
        return
    except Exception:
        pass
    import sys
    import types

    import antenv

    m = types.ModuleType("antenv.axon_hooks")
    m._h = None
    m.set_axon_ntff_profile_hook = lambda h: setattr(m, "_h", h)
    m.get_axon_ntff_profile_hook = lambda: m._h
    sys.modules["antenv.axon_hooks"] = m
    antenv.axon_hooks = m
    try:
        from trn_agent_boot.trn_boot import _ntff_profile_via_ctypes

        m._h = _ntff_profile_via_ctypes("/opt/axon/libaxon_pjrt.so")
    except Exception:
        pass


def kernel(**inputs):
    global _last_exec_time_ns, _last_results
    _ensure_axon_hooks()
    geoms, in_maps = _prep_inputs(**inputs)
    nc = _build_program(geoms)
    res = run_bass_kernel_spmd(nc, in_maps, core_ids=list(range(8)))
    _last_results = res
    _last_exec_time_ns = res.exec_time_ns
    out = np.stack([res.results[i]["out"] for i in range(8)], axis=0)
    return out.astype(np.float32)


# revision 5
# speedup vs baseline: 1.3033x; 1.3033x over previous
"""AdaptivePeriodicLayer Trainium2 kernel.

Strategy: data-parallel over batch (8 samples -> 8 NeuronCores), one SPMD
program with an 8-way tc.Switch on partition id. Each arm is specialized at
build time to that sample's 3 FFT-detected period geometries (computed on
host, like the reference's .item() round-trip). Convs are computed as
per-kernel-offset matmuls accumulated in PSUM over a zero-padded 2D canvas
held in SBUF; conv operands are bf16 (fp32 PSUM accumulation), everything
else fp32.
"""

import numpy as np

import concourse.bacc as bacc
import concourse.bass as bass
import concourse.mybir as mybir
from concourse.tile import TileContext
from concourse.bass_utils import run_bass_kernel_spmd

F32 = mybir.dt.float32
BF16 = mybir.dt.bfloat16
AF = mybir.ActivationFunctionType
AX = mybir.AxisListType

B, S, D, DFF, K = 8, 512, 128, 256, 3
NOFF = 35  # 1 + 9 + 25 conv taps across the three inception branches
WCHUNK = 7  # offsets per weight DMA chunk
NWCH = 5    # 35 / 7

_last_exec_time_ns = None
_last_results = None


def _offsets():
    offs = []
    for ksz in (1, 3, 5):
        q = (ksz - 1) // 2
        for kh in range(ksz):
            for kw in range(ksz):
                offs.append((q, kh, kw))
    return offs


OFFS = _offsets()


def _row_chunks(p, n):
    """Split the p x n image (h-major) into row groups of <=512 columns."""
    ch = max(1, min(p, 512 // n))
    out = []
    h0 = 0
    while h0 < p:
        rows = min(ch, p - h0)
        out.append((h0, rows))
        h0 += rows
    return out


def _build_program(geoms):
    """geoms: [8][3] list of (p, n) per core per pass."""
    nc = bacc.Bacc()

    xb = nc.declare_dram_parameter("xb", [S, D], F32, isOutput=False)
    w1 = nc.declare_dram_parameter("w1", [D, NOFF * 2 * 128], BF16, isOutput=False)
    w2 = nc.declare_dram_parameter("w2", [D, NOFF * 2 * 128], BF16, isOutput=False)
    b1 = nc.declare_dram_parameter("b1", [DFF, 1], F32, isOutput=False)
    eb = nc.declare_dram_parameter("eb", [D, 1], F32, isOutput=False)
    enh = nc.declare_dram_parameter("enh", [D, D], F32, isOutput=False)
    ident = nc.declare_dram_parameter("ident", [128, 128], F32, isOutput=False)
    aw1 = nc.declare_dram_parameter("aw1", [1, K * 2 * K], F32, isOutput=False)
    aw2 = nc.declare_dram_parameter("aw2", [1, 2 * K * K], F32, isOutput=False)
    ab1 = nc.declare_dram_parameter("ab1", [1, 2 * K], F32, isOutput=False)
    ab2 = nc.declare_dram_parameter("ab2", [1, K], F32, isOutput=False)
    pwd = nc.declare_dram_parameter("pw", [1, K], F32, isOutput=False)
    out_d = nc.declare_dram_parameter("out", [S, D], F32, isOutput=True)

    with TileContext(nc) as tc:
        with (
            tc.tile_pool(name="persist", bufs=1) as pp,
            tc.tile_pool(name="work", bufs=1) as wp,
            tc.tile_pool(name="psum", bufs=1, space="PSUM") as qp,
        ):
            # ---- persistent loads (identical data on every core) ----
            identS = pp.tile([128, 128], F32, tag="identS", name="identS")
            nc.sync.dma_start(out=identS, in_=ident[:, :])

            xall = pp.tile([128, S], F32, tag="xall", name="xall")
            for c in range(4):
                nc.sync.dma_start(out=xall[:, c * 128:(c + 1) * 128],
                                  in_=xb[c * 128:(c + 1) * 128, :])

            w1c = []
            for ci in range(NWCH):
                t = pp.tile([128, WCHUNK * 2 * 128], BF16, tag=f"w1c{ci}",
                            name=f"w1c{ci}")
                nc.sync.dma_start(
                    out=t, in_=w1[:, ci * WCHUNK * 256:(ci + 1) * WCHUNK * 256])
                w1c.append(t)

            b1S = pp.tile([128, 2], F32, tag="b1S", name="b1S")
            nc.sync.dma_start(out=b1S[:, 0:1], in_=b1[0:128, :])
            nc.sync.dma_start(out=b1S[:, 1:2], in_=b1[128:256, :])
            ebS = pp.tile([128, 1], F32, tag="ebS", name="ebS")
            nc.sync.dma_start(out=ebS, in_=eb[:, :])
            enhS = pp.tile([128, 128], F32, tag="enhS", name="enhS")
            nc.sync.dma_start(out=enhS, in_=enh[:, :])
            aw1S = pp.tile([1, K * 2 * K], F32, tag="aw1S", name="aw1S")
            nc.sync.dma_start(out=aw1S, in_=aw1[:, :])
            aw2S = pp.tile([1, 2 * K * K], F32, tag="aw2S", name="aw2S")
            nc.sync.dma_start(out=aw2S, in_=aw2[:, :])
            ab1S = pp.tile([1, 2 * K], F32, tag="ab1S", name="ab1S")
            nc.sync.dma_start(out=ab1S, in_=ab1[:, :])
            ab2S = pp.tile([1, K], F32, tag="ab2S", name="ab2S")
            nc.sync.dma_start(out=ab2S, in_=ab2[:, :])
            pwS = pp.tile([1, K], F32, tag="pwS", name="pwS")
            nc.sync.dma_start(out=pwS, in_=pwd[:, :])

            w2c = []
            for ci in range(NWCH):
                t = pp.tile([128, WCHUNK * 2 * 128], BF16, tag=f"w2c{ci}",
                            name=f"w2c{ci}")
                nc.sync.dma_start(
                    out=t, in_=w2[:, ci * WCHUNK * 256:(ci + 1) * WCHUNK * 256])
                w2c.append(t)

            # constants
            onesM = pp.tile([128, 1], F32, tag="onesM", name="onesM")
            nc.vector.memset(onesM, 1.0 / 128.0)
            ones1_128 = pp.tile([1, 128], F32, tag="ones1_128", name="ones1_128")
            nc.vector.memset(ones1_128, 1.0)

            # ---- transpose x: [S, D] -> xT [D, S] ----
            xT = pp.tile([128, S], F32, tag="xT", name="xT")
            for c in range(4):
                pt = qp.tile([128, 512], F32, tag="ps_small", bufs=2,
                             name=f"ptx{c}")
                nc.tensor.transpose(pt[:, 0:128],
                                    xall[:, c * 128:(c + 1) * 128], identS)
                nc.vector.tensor_copy(out=xT[:, c * 128:(c + 1) * 128],
                                      in_=pt[:, 0:128])

            def w1_lhsT(o, mc):
                ci, oi = divmod(o, WCHUNK)
                base = (oi * 2 + mc) * 128
                return w1c[ci][:, base:base + 128]

            def w2_lhsT(o, kc):
                ci, oi = divmod(o, WCHUNK)
                base = (oi * 2 + kc) * 128
                return w2c[ci][:, base:base + 128]

            def _transpose_po(core, k, po_k, poT_k):
                for c in range(4):
                    pt = qp.tile([128, 512], F32, tag="ps_small", bufs=2,
                                 name=f"ptp{k}{c}_{core}")
                    nc.tensor.transpose(pt[:, 0:128],
                                        po_k[:, c * 128:(c + 1) * 128],
                                        identS)
                    nc.vector.tensor_copy(
                        out=poT_k[:, c * 128:(c + 1) * 128], in_=pt[:, 0:128])

            def _fill_canvas(core, k, cv):
                p, n = geoms[core][k]
                nfull = S // p
                r = S - nfull * p
                src = xT[:, 0:nfull * p].rearrange("d (w h) -> d h w", h=p)
                nc.vector.tensor_copy(out=cv[:, 2:2 + p, 2:2 + nfull],
                                      in_=src)
                if r > 0:
                    tsrc = xT[:, nfull * p:S].rearrange("d (h w) -> d h w",
                                                        w=1)
                    nc.vector.tensor_copy(
                        out=cv[:, 2:2 + r, 2 + nfull:3 + nfull], in_=tsrc)

            def build_arm(core):
                # allocate per-pass tiles; memset canvases early so DVE works
                # while the weight DMAs stream in
                cvs, hcs = [], []
                for k in range(K):
                    p, n = geoms[core][k]
                    Hp, Wp = p + 4, n + 4
                    canvas = wp.tile([128, Hp * Wp], BF16, tag=f"canvas{k}",
                                     name=f"canvas{k}_{core}")
                    nc.gpsimd.memset(canvas, 0.0)
                    cvs.append(canvas.rearrange("d (h w) -> d h w", w=Wp))
                    if k == 0:
                        _fill_canvas(core, 0, cvs[0])
                    hk = []
                    for mc in range(2):
                        hc = wp.tile([128, Hp * Wp], BF16, tag=f"hc{k}_{mc}",
                                     name=f"hc{k}_{mc}_{core}")
                        nc.gpsimd.memset(hc, 0.0)
                        hk.append(hc.rearrange("d (h w) -> d h w", w=Wp))
                    hcs.append(hk)

                po, poT = [], []
                polastT = wp.tile([128, K], F32, tag="polast",
                                  name=f"polast_{core}")
                for k in range(K):
                    p, n = geoms[core][k]
                    L = p * n
                    nfull = S // p
                    r = S - nfull * p
                    chunks = _row_chunks(p, n)
                    assert len(chunks) <= 2, (p, n, chunks)
                    cv, hcv = cvs[k], hcs[k]

                    if k > 0:
                        _fill_canvas(core, k, cv)

                    # conv1: accumulate 35 taps per (Cout chunk, row chunk)
                    ps1 = [qp.tile([128, 1024], F32, tag=f"c1p{mc}",
                                   name=f"c1p{mc}_{core}_{k}")
                           for mc in range(2)]
                    for mc in range(2):
                        for o, (q, kh, kw) in enumerate(OFFS):
                            for j, (h0, rows) in enumerate(chunks):
                                rhs = cv[:, 2 - q + kh + h0:
                                         2 - q + kh + h0 + rows,
                                         2 - q + kw:2 - q + kw + n]
                                nc.tensor.matmul(
                                    ps1[mc][:, j * 512:j * 512 + rows * n],
                                    w1_lhsT(o, mc), rhs,
                                    start=(o == 0), stop=(o == NOFF - 1))
                        # gelu(conv1 + b1) written into padded h-canvas
                        for j, (h0, rows) in enumerate(chunks):
                            nc.scalar.activation(
                                out=hcv[mc][:, 2 + h0:2 + h0 + rows, 2:2 + n],
                                in_=ps1[mc][:, j * 512:j * 512 + rows * n],
                                func=AF.Gelu, bias=b1S[:, mc:mc + 1])

                    # conv2: contract over 256 channels (2 K-chunks) x 35 taps
                    ps2 = qp.tile([128, 1024], F32, tag="c2p",
                                  name=f"c2p_{core}_{k}")
                    for kc in range(2):
                        for o, (q, kh, kw) in enumerate(OFFS):
                            for j, (h0, rows) in enumerate(chunks):
                                rhs = hcv[kc][:, 2 - q + kh + h0:
                                              2 - q + kh + h0 + rows,
                                              2 - q + kw:2 - q + kw + n]
                                nc.tensor.matmul(
                                    ps2[:, j * 512:j * 512 + rows * n],
                                    w2_lhsT(o, kc), rhs,
                                    start=(kc == 0 and o == 0),
                                    stop=(kc == 1 and o == NOFF - 1))

                    # conv2 out back to SBUF, contiguous h-major
                    c2 = wp.tile([128, 1024], F32, tag="c2",
                                 name=f"c2_{core}_{k}")
                    for j, (h0, rows) in enumerate(chunks):
                        nc.vector.tensor_copy(
                            out=c2[:, h0 * n:h0 * n + rows * n],
                            in_=ps2[:, j * 512:j * 512 + rows * n])

                    # period enhancement on the first S sequence positions
                    ps3 = qp.tile([128, 512], F32, tag="ps_small", bufs=2,
                                  name=f"ps3_{core}_{k}")
                    c2v = c2[:, 0:L].rearrange("d (h w) -> d w h", w=n)
                    nc.tensor.matmul(ps3[:, 0:nfull * p], enhS,
                                     c2v[:, 0:nfull, :],
                                     start=True, stop=(r == 0))
                    if r > 0:
                        nc.tensor.matmul(ps3[:, nfull * p:S], enhS,
                                         c2v[:, nfull:nfull + 1, 0:r],
                                         start=False, stop=True)
                    po_k = wp.tile([128, S], F32, tag=f"po{k}",
                                   name=f"po{k}_{core}")
                    nc.scalar.activation(out=po_k, in_=ps3[:, 0:S],
                                         func=AF.Gelu, bias=ebS[:, 0:1])
                    po.append(po_k)
                    nc.vector.tensor_copy(out=polastT[:, k:k + 1],
                                          in_=po_k[:, S - 1:S])
                    # eager transpose to [s, d] chunks (overlaps next pass;
                    # pass 2's is deferred to overlap the attention chain)
                    poT_k = wp.tile([128, S], F32, tag=f"poT{k}",
                                    name=f"poT{k}_{core}")
                    if k < 2:
                        _transpose_po(core, k, po_k, poT_k)
                    poT.append(poT_k)

                # ---- combination weights (free-dim chain on partition 0) ----
                lsfP = qp.tile([1, K], F32, tag="ps_small", bufs=2,
                               name=f"lsfP_{core}")
                nc.tensor.matmul(lsfP, onesM, polastT, start=True, stop=True)
                lsfS = wp.tile([1, K], F32, tag="lsfS", name=f"lsfS_{core}")
                nc.vector.tensor_copy(out=lsfS, in_=lsfP)

                t6 = wp.tile([1, 2 * K], F32, tag="t6", name=f"t6_{core}")
                tt6 = wp.tile([1, 2 * K], F32, tag="tt6", name=f"tt6_{core}")
                nc.vector.tensor_scalar_mul(t6, aw1S[:, 0:6], lsfS[:, 0:1])
                nc.vector.tensor_scalar_mul(tt6, aw1S[:, 6:12], lsfS[:, 1:2])
                nc.vector.tensor_add(t6, t6, tt6)
                nc.vector.tensor_scalar_mul(tt6, aw1S[:, 12:18], lsfS[:, 2:3])
                nc.vector.tensor_add(t6, t6, tt6)
                nc.vector.tensor_add(t6, t6, ab1S)
                nc.vector.tensor_scalar_max(t6, t6, 0.0)

                a3 = wp.tile([1, K], F32, tag="a3", name=f"a3_{core}")
                tt3 = wp.tile([1, K], F32, tag="tt3", name=f"tt3_{core}")
                nc.vector.tensor_scalar_mul(a3, aw2S[:, 0:3], t6[:, 0:1])
                for i in range(1, 6):
                    nc.vector.tensor_scalar_mul(tt3, aw2S[:, 3 * i:3 * i + 3],
                                                t6[:, i:i + 1])
                    nc.vector.tensor_add(a3, a3, tt3)
                nc.vector.tensor_add(a3, a3, ab2S)

                e1 = wp.tile([1, K], F32, tag="e1", name=f"e1_{core}")
                nc.scalar.activation(out=e1, in_=a3, func=AF.Exp)
                s1 = wp.tile([1, 1], F32, tag="s1", name=f"s1_{core}")
                nc.vector.reduce_sum(out=s1, in_=e1, axis=AX.X)
                r1 = wp.tile([1, 1], F32, tag="r1", name=f"r1_{core}")
                nc.vector.reciprocal(r1, s1)
                z = wp.tile([1, K], F32, tag="z", name=f"z_{core}")
                nc.vector.tensor_scalar_mul(z, e1, r1[:, 0:1])
                nc.vector.tensor_mul(z, z, pwS)
                e2 = wp.tile([1, K], F32, tag="e2", name=f"e2_{core}")
                nc.scalar.activation(out=e2, in_=z, func=AF.Exp)
                s2 = wp.tile([1, 1], F32, tag="s2", name=f"s2_{core}")
                nc.vector.reduce_sum(out=s2, in_=e2, axis=AX.X)
                r2 = wp.tile([1, 1], F32, tag="r2", name=f"r2_{core}")
                nc.vector.reciprocal(r2, s2)
                cwrow = wp.tile([1, K], F32, tag="cwrow", name=f"cwrow_{core}")
                nc.vector.tensor_scalar_mul(cwrow, e2, r2[:, 0:1])

                cwbP = qp.tile([128, K], F32, tag="ps_small", bufs=2,
                               name=f"cwbP_{core}")
                nc.tensor.matmul(cwbP, ones1_128, cwrow, start=True, stop=True)
                cwS = wp.tile([128, K], F32, tag="cwS", name=f"cwS_{core}")
                nc.vector.tensor_copy(out=cwS, in_=cwbP)

                _transpose_po(core, 2, po[2], poT[2])

                # ---- weighted combine + residual, already in [s, d] ----
                accT = wp.tile([128, S], F32, tag="accT", name=f"accT_{core}")
                tmpT = wp.tile([128, S], F32, tag="tmpT", name=f"tmpT_{core}")
                nc.vector.tensor_scalar_mul(accT, poT[0], cwS[:, 0:1])
                nc.vector.tensor_scalar_mul(tmpT, poT[1], cwS[:, 1:2])
                nc.vector.tensor_add(accT, accT, tmpT)
                nc.vector.tensor_scalar_mul(tmpT, poT[2], cwS[:, 2:3])
                nc.vector.tensor_add(accT, accT, tmpT)
                nc.vector.tensor_add(accT, accT, xall)

                for c in range(4):
                    nc.sync.dma_start(out=out_d[c * 128:(c + 1) * 128, :],
                                      in_=accT[:, c * 128:(c + 1) * 128])

            pid = nc.partition_id(engines=mybir.ALL_ENGINES)
            for core in tc.Switch(pid, 8):
                build_arm(core)

    nc.finalize()
    return nc


def _prep_inputs(x, conv1_w, conv1_b, conv2_w, conv2_b, attn_w1, attn_b1,
                 attn_w2, attn_b2, enh_w, enh_b):
    x = np.asarray(x, dtype=np.float32)
    conv1_w = [np.asarray(w, dtype=np.float32) for w in conv1_w]
    conv1_b = [np.asarray(w, dtype=np.float32) for w in conv1_b]
    conv2_w = [np.asarray(w, dtype=np.float32) for w in conv2_w]
    conv2_b = [np.asarray(w, dtype=np.float32) for w in conv2_b]
    enh_w = np.asarray(enh_w, dtype=np.float32)
    enh_b = np.asarray(enh_b, dtype=np.float32)

    # FFT period detection on host (reference does the same via .item())
    xf = np.abs(np.fft.rfft(x, axis=1)).mean(-1).astype(np.float32)
    xf[:, 0] = 0.0
    top = np.argsort(-xf, axis=1, kind="stable")[:, :K]
    geoms, pws = [], []
    for b in range(B):
        g = []
        for k in range(K):
            f = int(top[b, k])
            p = S if f == 0 else max(1, S // f)
            pad = (-S) % p
            g.append((p, (S + pad) // p))
        geoms.append(g)
        wv = xf[b, top[b]].astype(np.float64)
        ev = np.exp(wv - wv.max())
        pws.append((ev / ev.sum()).astype(np.float32).reshape(1, K))

    # conv weights -> per-tap transposed matmul operands, pre-scaled by 1/3
    w1A = np.zeros((128, NOFF, 2, 128), dtype=np.float32)
    w2A = np.zeros((128, NOFF, 2, 128), dtype=np.float32)
    o = 0
    for br, ksz in enumerate((1, 3, 5)):
        for kh in range(ksz):
            for kw in range(ksz):
                wt1 = conv1_w[br][:, :, kh, kw] / 3.0   # [256(out), 128(in)]
                w1A[:, o, 0, :] = wt1[0:128, :].T
                w1A[:, o, 1, :] = wt1[128:256, :].T
                wt2 = conv2_w[br][:, :, kh, kw] / 3.0   # [128(out), 256(in)]
                w2A[:, o, 0, :] = wt2[:, 0:128].T
                w2A[:, o, 1, :] = wt2[:, 128:256].T
                o += 1
    import ml_dtypes
    w1A = w1A.reshape(128, NOFF * 2 * 128).astype(ml_dtypes.bfloat16)
    w2A = w2A.reshape(128, NOFF * 2 * 128).astype(ml_dtypes.bfloat16)
    b1m = (sum(conv1_b) / 3.0).reshape(DFF, 1).astype(np.float32)
    b2m = (sum(conv2_b) / 3.0).astype(np.float32)
    ebv = (b2m @ enh_w + enh_b).reshape(D, 1).astype(np.float32)

    shared = dict(
        w1=w1A, w2=w2A, b1=b1m, eb=ebv,
        enh=np.ascontiguousarray(enh_w),
        ident=np.eye(128, dtype=np.float32),
        aw1=np.asarray(attn_w1, dtype=np.float32).reshape(1, K * 2 * K),
        aw2=np.asarray(attn_w2, dtype=np.float32).reshape(1, 2 * K * K),
        ab1=np.asarray(attn_b1, dtype=np.float32).reshape(1, 2 * K),
        ab2=np.asarray(attn_b2, dtype=np.float32).reshape(1, K),
    )
    in_maps = []
    for b in range(B):
        m = dict(shared)
        m["xb"] = np.ascontiguousarray(x[b])
        m["pw"] = pws[b]
        in_maps.append(m)
    return geoms, in_maps


def _ensure_axon_hooks():
    """bass_utils' trace path imports antenv.axon_hooks unconditionally;
    the container's antenv lacks it. Provide it, registering the real
    ctypes NTFF hook when available so tracing works."""
    try:
        import antenv.axon_hooks  # noqa: F401
        return
    except Exception:
        pass
    import sys
    import types

    import antenv

    m = types.ModuleType("antenv.axon_hooks")
    m._h = None
    m.set_axon_ntff_profile_hook = lambda h: setattr(m, "_h", h)
    m.get_axon_ntff_profile_hook = lambda: m._h
    sys.modules["antenv.axon_hooks"] = m
    antenv.axon_hooks = m
    try:
        from trn_agent_boot.trn_boot import _ntff_profile_via_ctypes

        m._h = _ntff_profile_via_ctypes("/opt/axon/libaxon_pjrt.so")
    except Exception:
        pass


def kernel(**inputs):
    global _last_exec_time_ns, _last_results
    _ensure_axon_hooks()
    geoms, in_maps = _prep_inputs(**inputs)
    nc = _build_program(geoms)
    res = run_bass_kernel_spmd(nc, in_maps, core_ids=list(range(8)))
    _last_results = res
    _last_exec_time_ns = res.exec_time_ns
    out = np.stack([res.results[i]["out"] for i in range(8)], axis=0)
    return out.astype(np.float32)


# revision 7
# speedup vs baseline: 1.3324x; 1.0223x over previous
"""AdaptivePeriodicLayer Trainium2 kernel.

Strategy: data-parallel over batch (8 samples -> 8 NeuronCores), one SPMD
program with an 8-way tc.Switch on partition id. Each arm is specialized at
build time to that sample's 3 FFT-detected period geometries (computed on
host, like the reference's .item() round-trip). Convs are computed as
per-kernel-offset matmuls accumulated in PSUM over a zero-padded 2D canvas
held in SBUF; conv operands are bf16 (fp32 PSUM accumulation), everything
else fp32.
"""

import numpy as np

import concourse.bacc as bacc
import concourse.bass as bass
import concourse.mybir as mybir
from concourse.tile import TileContext
from concourse.bass_utils import run_bass_kernel_spmd

F32 = mybir.dt.float32
BF16 = mybir.dt.bfloat16
AF = mybir.ActivationFunctionType
AX = mybir.AxisListType

B, S, D, DFF, K = 8, 512, 128, 256, 3
NOFF = 35  # 1 + 9 + 25 conv taps across the three inception branches
WCHUNK = 7  # offsets per weight DMA chunk
NWCH = 5    # 35 / 7

_last_exec_time_ns = None
_last_results = None


def _offsets():
    offs = []
    for ksz in (1, 3, 5):
        q = (ksz - 1) // 2
        for kh in range(ksz):
            for kw in range(ksz):
                offs.append((q, kh, kw))
    return offs


OFFS = _offsets()


def _row_chunks(p, n):
    """Split the p x n image (h-major) into row groups of <=512 columns."""
    ch = max(1, min(p, 512 // n))
    out = []
    h0 = 0
    while h0 < p:
        rows = min(ch, p - h0)
        out.append((h0, rows))
        h0 += rows
    return out


def _build_program(geoms):
    """geoms: [8][3] list of (p, n) per core per pass."""
    nc = bacc.Bacc()

    xb = nc.declare_dram_parameter("xb", [S, D], F32, isOutput=False)
    w1 = nc.declare_dram_parameter("w1", [D, NOFF * 2 * 128], BF16, isOutput=False)
    w2 = nc.declare_dram_parameter("w2", [D, NOFF * 2 * 128], BF16, isOutput=False)
    b1 = nc.declare_dram_parameter("b1", [DFF, 1], F32, isOutput=False)
    eb = nc.declare_dram_parameter("eb", [D, 1], F32, isOutput=False)
    enh = nc.declare_dram_parameter("enh", [D, D], F32, isOutput=False)
    ident = nc.declare_dram_parameter("ident", [128, 128], F32, isOutput=False)
    aw1 = nc.declare_dram_parameter("aw1", [1, K * 2 * K], F32, isOutput=False)
    aw2 = nc.declare_dram_parameter("aw2", [1, 2 * K * K], F32, isOutput=False)
    ab1 = nc.declare_dram_parameter("ab1", [1, 2 * K], F32, isOutput=False)
    ab2 = nc.declare_dram_parameter("ab2", [1, K], F32, isOutput=False)
    pwd = nc.declare_dram_parameter("pw", [1, K], F32, isOutput=False)
    out_d = nc.declare_dram_parameter("out", [S, D], F32, isOutput=True)

    with TileContext(nc) as tc:
        with (
            tc.tile_pool(name="persist", bufs=1) as pp,
            tc.tile_pool(name="work", bufs=1) as wp,
            tc.tile_pool(name="psum", bufs=1, space="PSUM") as qp,
        ):
            # ---- persistent loads (identical data on every core) ----
            identS = pp.tile([128, 128], F32, tag="identS", name="identS")
            nc.sync.dma_start(out=identS, in_=ident[:, :])

            xall = pp.tile([128, S], F32, tag="xall", name="xall")
            for c in range(4):
                nc.sync.dma_start(out=xall[:, c * 128:(c + 1) * 128],
                                  in_=xb[c * 128:(c + 1) * 128, :])

            w1c = []
            for ci in range(NWCH):
                t = pp.tile([128, WCHUNK * 2 * 128], BF16, tag=f"w1c{ci}",
                            name=f"w1c{ci}")
                nc.sync.dma_start(
                    out=t, in_=w1[:, ci * WCHUNK * 256:(ci + 1) * WCHUNK * 256])
                w1c.append(t)

            b1S = pp.tile([128, 2], F32, tag="b1S", name="b1S")
            nc.sync.dma_start(out=b1S[:, 0:1], in_=b1[0:128, :])
            nc.sync.dma_start(out=b1S[:, 1:2], in_=b1[128:256, :])
            ebS = pp.tile([128, 1], F32, tag="ebS", name="ebS")
            nc.sync.dma_start(out=ebS, in_=eb[:, :])
            enhS = pp.tile([128, 128], F32, tag="enhS", name="enhS")
            nc.sync.dma_start(out=enhS, in_=enh[:, :])
            aw1S = pp.tile([1, K * 2 * K], F32, tag="aw1S", name="aw1S")
            nc.sync.dma_start(out=aw1S, in_=aw1[:, :])
            aw2S = pp.tile([1, 2 * K * K], F32, tag="aw2S", name="aw2S")
            nc.sync.dma_start(out=aw2S, in_=aw2[:, :])
            ab1S = pp.tile([1, 2 * K], F32, tag="ab1S", name="ab1S")
            nc.sync.dma_start(out=ab1S, in_=ab1[:, :])
            ab2S = pp.tile([1, K], F32, tag="ab2S", name="ab2S")
            nc.sync.dma_start(out=ab2S, in_=ab2[:, :])
            pwS = pp.tile([1, K], F32, tag="pwS", name="pwS")
            nc.sync.dma_start(out=pwS, in_=pwd[:, :])

            w2c = []
            for ci in range(NWCH):
                t = pp.tile([128, WCHUNK * 2 * 128], BF16, tag=f"w2c{ci}",
                            name=f"w2c{ci}")
                nc.sync.dma_start(
                    out=t, in_=w2[:, ci * WCHUNK * 256:(ci + 1) * WCHUNK * 256])
                w2c.append(t)

            # constants
            onesM = pp.tile([128, 1], F32, tag="onesM", name="onesM")
            nc.vector.memset(onesM, 1.0 / 128.0)
            ones1_128 = pp.tile([1, 128], F32, tag="ones1_128", name="ones1_128")
            nc.vector.memset(ones1_128, 1.0)

            # ---- transpose x: [S, D] -> xT [D, S] ----
            xT = pp.tile([128, S], F32, tag="xT", name="xT")
            for c in range(4):
                pt = qp.tile([128, 512], F32, tag="ps_small", bufs=2,
                             name=f"ptx{c}")
                nc.tensor.transpose(pt[:, 0:128],
                                    xall[:, c * 128:(c + 1) * 128], identS)
                nc.vector.tensor_copy(out=xT[:, c * 128:(c + 1) * 128],
                                      in_=pt[:, 0:128])

            def w1_lhsT(o, mc):
                ci, oi = divmod(o, WCHUNK)
                base = (oi * 2 + mc) * 128
                return w1c[ci][:, base:base + 128]

            def w2_lhsT(o, kc):
                ci, oi = divmod(o, WCHUNK)
                base = (oi * 2 + kc) * 128
                return w2c[ci][:, base:base + 128]

            def _transpose_po(core, k, po_k, poT_k):
                for c in range(4):
                    pt = qp.tile([128, 512], F32, tag="ps_small", bufs=2,
                                 name=f"ptp{k}{c}_{core}")
                    nc.tensor.transpose(pt[:, 0:128],
                                        po_k[:, c * 128:(c + 1) * 128],
                                        identS)
                    nc.vector.tensor_copy(
                        out=poT_k[:, c * 128:(c + 1) * 128], in_=pt[:, 0:128])

            def _fill_canvas(core, k, cv):
                p, n = geoms[core][k]
                nfull = S // p
                r = S - nfull * p
                src = xT[:, 0:nfull * p].rearrange("d (w h) -> d h w", h=p)
                nc.vector.tensor_copy(out=cv[:, 2:2 + p, 2:2 + nfull],
                                      in_=src)
                if r > 0:
                    tsrc = xT[:, nfull * p:S].rearrange("d (h w) -> d h w",
                                                        w=1)
                    nc.vector.tensor_copy(
                        out=cv[:, 2:2 + r, 2 + nfull:3 + nfull], in_=tsrc)

            def build_arm(core):
                # allocate per-pass tiles; memset canvases early so DVE works
                # while the weight DMAs stream in
                cvs, hcs = [], []
                for k in range(K):
                    p, n = geoms[core][k]
                    Hp, Wp = p + 4, n + 4
                    canvas = wp.tile([128, Hp * Wp], BF16, tag=f"canvas{k}",
                                     name=f"canvas{k}_{core}")
                    nc.gpsimd.memset(canvas, 0.0)
                    cvs.append(canvas.rearrange("d (h w) -> d h w", w=Wp))
                    if k == 0:
                        _fill_canvas(core, 0, cvs[0])
                    hk = []
                    for mc in range(2):
                        hc = wp.tile([128, Hp * Wp], BF16, tag=f"hc{k}_{mc}",
                                     name=f"hc{k}_{mc}_{core}")
                        nc.gpsimd.memset(hc, 0.0)
                        hk.append(hc.rearrange("d (h w) -> d h w", w=Wp))
                    hcs.append(hk)

                po, poT = [], []
                c2s, ps3s = {}, {}
                polastT = wp.tile([128, K], F32, tag="polast",
                                  name=f"polast_{core}")

                def finish_pass(k):
                    p, n = geoms[core][k]
                    L = p * n
                    nfull = S // p
                    r = S - nfull * p
                    c2 = c2s[k]
                    ps3 = qp.tile([128, 512], F32, tag="ps_small", bufs=2,
                                  name=f"ps3_{core}_{k}")
                    c2v = c2[:, 0:L].rearrange("d (h w) -> d w h", w=n)
                    nc.tensor.matmul(ps3[:, 0:nfull * p], enhS,
                                     c2v[:, 0:nfull, :],
                                     start=True, stop=(r == 0))
                    if r > 0:
                        nc.tensor.matmul(ps3[:, nfull * p:S], enhS,
                                         c2v[:, nfull:nfull + 1, 0:r],
                                         start=False, stop=True)
                    po_k = wp.tile([128, S], F32, tag=f"po{k}",
                                   name=f"po{k}_{core}")
                    nc.scalar.activation(out=po_k, in_=ps3[:, 0:S],
                                         func=AF.Gelu, bias=ebS[:, 0:1])
                    po.append(po_k)
                    nc.vector.tensor_copy(out=polastT[:, k:k + 1],
                                          in_=po_k[:, S - 1:S])
                    poT_k = wp.tile([128, S], F32, tag=f"poT{k}",
                                    name=f"poT{k}_{core}")
                    if k < 2:
                        _transpose_po(core, k, po_k, poT_k)
                    poT.append(poT_k)

                for k in range(K):
                    p, n = geoms[core][k]
                    L = p * n
                    nfull = S // p
                    r = S - nfull * p
                    chunks = _row_chunks(p, n)
                    assert len(chunks) <= 2, (p, n, chunks)
                    cv, hcv = cvs[k], hcs[k]

                    if k > 0:
                        _fill_canvas(core, k, cv)

                    # conv1: accumulate 35 taps per (Cout chunk, row chunk)
                    ps1 = [qp.tile([128, 1024], F32, tag=f"c1p{mc}",
                                   name=f"c1p{mc}_{core}_{k}")
                           for mc in range(2)]
                    for mc in range(2):
                        for o, (q, kh, kw) in enumerate(OFFS):
                            for j, (h0, rows) in enumerate(chunks):
                                rhs = cv[:, 2 - q + kh + h0:
                                         2 - q + kh + h0 + rows,
                                         2 - q + kw:2 - q + kw + n]
                                nc.tensor.matmul(
                                    ps1[mc][:, j * 512:j * 512 + rows * n],
                                    w1_lhsT(o, mc), rhs,
                                    start=(o == 0), stop=(o == NOFF - 1))
                        # gelu(conv1 + b1) written into padded h-canvas
                        for j, (h0, rows) in enumerate(chunks):
                            nc.scalar.activation(
                                out=hcv[mc][:, 2 + h0:2 + h0 + rows, 2:2 + n],
                                in_=ps1[mc][:, j * 512:j * 512 + rows * n],
                                func=AF.Gelu, bias=b1S[:, mc:mc + 1])

                    if k > 0:
                        finish_pass(k - 1)

                    # conv2: contract over 256 channels (2 K-chunks) x 35 taps
                    ps2 = qp.tile([128, 1024], F32, tag="c2p",
                                  name=f"c2p_{core}_{k}")
                    for kc in range(2):
                        for o, (q, kh, kw) in enumerate(OFFS):
                            for j, (h0, rows) in enumerate(chunks):
                                rhs = hcv[kc][:, 2 - q + kh + h0:
                                              2 - q + kh + h0 + rows,
                                              2 - q + kw:2 - q + kw + n]
                                nc.tensor.matmul(
                                    ps2[:, j * 512:j * 512 + rows * n],
                                    w2_lhsT(o, kc), rhs,
                                    start=(kc == 0 and o == 0),
                                    stop=(kc == 1 and o == NOFF - 1))

                    # conv2 out back to SBUF, contiguous h-major
                    c2 = wp.tile([128, 1024], F32, tag="c2", bufs=2,
                                 name=f"c2_{core}_{k}")
                    for j, (h0, rows) in enumerate(chunks):
                        nc.vector.tensor_copy(
                            out=c2[:, h0 * n:h0 * n + rows * n],
                            in_=ps2[:, j * 512:j * 512 + rows * n])
                    c2s[k] = c2

                finish_pass(2)

                # ---- combination weights (free-dim chain on partition 0) ----
                lsfP = qp.tile([1, K], F32, tag="ps_small", bufs=2,
                               name=f"lsfP_{core}")
                nc.tensor.matmul(lsfP, onesM, polastT, start=True, stop=True)
                lsfS = wp.tile([1, K], F32, tag="lsfS", name=f"lsfS_{core}")
                nc.vector.tensor_copy(out=lsfS, in_=lsfP)

                t6 = wp.tile([1, 2 * K], F32, tag="t6", name=f"t6_{core}")
                tt6 = wp.tile([1, 2 * K], F32, tag="tt6", name=f"tt6_{core}")
                nc.vector.tensor_scalar_mul(t6, aw1S[:, 0:6], lsfS[:, 0:1])
                nc.vector.tensor_scalar_mul(tt6, aw1S[:, 6:12], lsfS[:, 1:2])
                nc.vector.tensor_add(t6, t6, tt6)
                nc.vector.tensor_scalar_mul(tt6, aw1S[:, 12:18], lsfS[:, 2:3])
                nc.vector.tensor_add(t6, t6, tt6)
                nc.vector.tensor_add(t6, t6, ab1S)
                nc.vector.tensor_scalar_max(t6, t6, 0.0)

                a3 = wp.tile([1, K], F32, tag="a3", name=f"a3_{core}")
                tt3 = wp.tile([1, K], F32, tag="tt3", name=f"tt3_{core}")
                nc.vector.tensor_scalar_mul(a3, aw2S[:, 0:3], t6[:, 0:1])
                for i in range(1, 6):
                    nc.vector.tensor_scalar_mul(tt3, aw2S[:, 3 * i:3 * i + 3],
                                                t6[:, i:i + 1])
                    nc.vector.tensor_add(a3, a3, tt3)
                nc.vector.tensor_add(a3, a3, ab2S)

                e1 = wp.tile([1, K], F32, tag="e1", name=f"e1_{core}")
                nc.scalar.activation(out=e1, in_=a3, func=AF.Exp)
                s1 = wp.tile([1, 1], F32, tag="s1", name=f"s1_{core}")
                nc.vector.reduce_sum(out=s1, in_=e1, axis=AX.X)
                r1 = wp.tile([1, 1], F32, tag="r1", name=f"r1_{core}")
                nc.vector.reciprocal(r1, s1)
                z = wp.tile([1, K], F32, tag="z", name=f"z_{core}")
                nc.vector.tensor_scalar_mul(z, e1, r1[:, 0:1])
                nc.vector.tensor_mul(z, z, pwS)
                e2 = wp.tile([1, K], F32, tag="e2", name=f"e2_{core}")
                nc.scalar.activation(out=e2, in_=z, func=AF.Exp)
                s2 = wp.tile([1, 1], F32, tag="s2", name=f"s2_{core}")
                nc.vector.reduce_sum(out=s2, in_=e2, axis=AX.X)
                r2 = wp.tile([1, 1], F32, tag="r2", name=f"r2_{core}")
                nc.vector.reciprocal(r2, s2)
                cwrow = wp.tile([1, K], F32, tag="cwrow", name=f"cwrow_{core}")
                nc.vector.tensor_scalar_mul(cwrow, e2, r2[:, 0:1])

                cwbP = qp.tile([128, K], F32, tag="ps_small", bufs=2,
                               name=f"cwbP_{core}")
                nc.tensor.matmul(cwbP, ones1_128, cwrow, start=True, stop=True)
                cwS = wp.tile([128, K], F32, tag="cwS", name=f"cwS_{core}")
                nc.vector.tensor_copy(out=cwS, in_=cwbP)

                _transpose_po(core, 2, po[2], poT[2])
                # ---- weighted combine + residual, already in [s, d] ----
                accT = wp.tile([128, S], F32, tag="accT", name=f"accT_{core}")
                q1T = wp.tile([128, S], F32, tag="q1T", name=f"q1T_{core}")
                q2T = wp.tile([128, S], F32, tag="q2T", name=f"q2T_{core}")
                nc.scalar.activation(out=q1T, in_=poT[1], func=AF.Copy,
                                     scale=cwS[:, 1:2])
                nc.scalar.activation(out=q2T, in_=poT[2], func=AF.Copy,
                                     scale=cwS[:, 2:3])
                nc.vector.tensor_scalar_mul(accT, poT[0], cwS[:, 0:1])
                nc.vector.tensor_add(accT, accT, xall)
                nc.vector.tensor_add(accT, accT, q1T)
                nc.vector.tensor_add(accT, accT, q2T)

                nc.sync.dma_start(
                    out=out_d.rearrange("(c sp) d -> sp c d", sp=128),
                    in_=accT.rearrange("sp (c d) -> sp c d", d=128))

            pid = nc.partition_id(engines=mybir.ALL_ENGINES)
            for core in tc.Switch(pid, 8):
                build_arm(core)

    nc.finalize()
    return nc


def _prep_inputs(x, conv1_w, conv1_b, conv2_w, conv2_b, attn_w1, attn_b1,
                 attn_w2, attn_b2, enh_w, enh_b):
    x = np.asarray(x, dtype=np.float32)
    conv1_w = [np.asarray(w, dtype=np.float32) for w in conv1_w]
    conv1_b = [np.asarray(w, dtype=np.float32) for w in conv1_b]
    conv2_w = [np.asarray(w, dtype=np.float32) for w in conv2_w]
    conv2_b = [np.asarray(w, dtype=np.float32) for w in conv2_b]
    enh_w = np.asarray(enh_w, dtype=np.float32)
    enh_b = np.asarray(enh_b, dtype=np.float32)

    # FFT period detection on host (reference does the same via .item())
    xf = np.abs(np.fft.rfft(x, axis=1)).mean(-1).astype(np.float32)
    xf[:, 0] = 0.0
    top = np.argsort(-xf, axis=1, kind="stable")[:, :K]
    geoms, pws = [], []
    for b in range(B):
        g = []
        for k in range(K):
            f = int(top[b, k])
            p = S if f == 0 else max(1, S // f)
            pad = (-S) % p
            g.append((p, (S + pad) // p))
        geoms.append(g)
        wv = xf[b, top[b]].astype(np.float64)
        ev = np.exp(wv - wv.max())
        pws.append((ev / ev.sum()).astype(np.float32).reshape(1, K))

    # conv weights -> per-tap transposed matmul operands, pre-scaled by 1/3
    w1A = np.zeros((128, NOFF, 2, 128), dtype=np.float32)
    w2A = np.zeros((128, NOFF, 2, 128), dtype=np.float32)
    o = 0
    for br, ksz in enumerate((1, 3, 5)):
        for kh in range(ksz):
            for kw in range(ksz):
                wt1 = conv1_w[br][:, :, kh, kw] / 3.0   # [256(out), 128(in)]
                w1A[:, o, 0, :] = wt1[0:128, :].T
                w1A[:, o, 1, :] = wt1[128:256, :].T
                wt2 = conv2_w[br][:, :, kh, kw] / 3.0   # [128(out), 256(in)]
                w2A[:, o, 0, :] = wt2[:, 0:128].T
                w2A[:, o, 1, :] = wt2[:, 128:256].T
                o += 1
    import ml_dtypes
    w1A = w1A.reshape(128, NOFF * 2 * 128).astype(ml_dtypes.bfloat16)
    w2A = w2A.reshape(128, NOFF * 2 * 128).astype(ml_dtypes.bfloat16)
    b1m = (sum(conv1_b) / 3.0).reshape(DFF, 1).astype(np.float32)
    b2m = (sum(conv2_b) / 3.0).astype(np.float32)
    ebv = (b2m @ enh_w + enh_b).reshape(D, 1).astype(np.float32)

    shared = dict(
        w1=w1A, w2=w2A, b1=b1m, eb=ebv,
        enh=np.ascontiguousarray(enh_w),
        ident=np.eye(128, dtype=np.float32),
        aw1=np.asarray(attn_w1, dtype=np.float32).reshape(1, K * 2 * K),
        aw2=np.asarray(attn_w2, dtype=np.float32).reshape(1, 2 * K * K),
        ab1=np.asarray(attn_b1, dtype=np.float32).reshape(1, 2 * K),
        ab2=np.asarray(attn_b2, dtype=np.float32).reshape(1, K),
    )
    in_maps = []
    for b in range(B):
        m = dict(shared)
        m["xb"] = np.ascontiguousarray(x[b])
        m["pw"] = pws[b]
        in_maps.append(m)
    return geoms, in_maps


def _ensure_axon_hooks():
    """bass_utils' trace path imports antenv.axon_hooks unconditionally;
    the container's antenv lacks it. Provide it, registering the real
    ctypes NTFF hook when available so tracing works."""
    try:
        import antenv.axon_hooks  # noqa: F401
        return
    except Exception:
        pass
    import sys
    import types

    import antenv

    m = types.ModuleType("antenv.axon_hooks")
    m._h = None
    m.set_axon_ntff_profile_hook = lambda h: setattr(m, "_h", h)
    m.get_axon_ntff_profile_hook = lambda: m._h
    sys.modules["antenv.axon_hooks"] = m
    antenv.axon_hooks = m
    try:
        from trn_agent_boot.trn_boot import _ntff_profile_via_ctypes

        m._h = _ntff_profile_via_ctypes("/opt/axon/libaxon_pjrt.so")
    except Exception:
        pass


def kernel(**inputs):
    global _last_exec_time_ns, _last_results
    _ensure_axon_hooks()
    geoms, in_maps = _prep_inputs(**inputs)
    nc = _build_program(geoms)
    res = run_bass_kernel_spmd(nc, in_maps, core_ids=list(range(8)))
    _last_results = res
    _last_exec_time_ns = res.exec_time_ns
    out = np.stack([res.results[i]["out"] for i in range(8)], axis=0)
    return out.astype(np.float32)


# revision 9
# speedup vs baseline: 1.3484x; 1.0120x over previous
"""AdaptivePeriodicLayer Trainium2 kernel.

Strategy: data-parallel over batch (8 samples -> 8 NeuronCores), one SPMD
program with an 8-way tc.Switch on partition id. Each arm is specialized at
build time to that sample's 3 FFT-detected period geometries (computed on
host, like the reference's .item() round-trip). Convs are computed as
per-kernel-offset matmuls accumulated in PSUM over a zero-padded 2D canvas
held in SBUF; conv operands are bf16 (fp32 PSUM accumulation), everything
else fp32.
"""

import numpy as np

import concourse.bacc as bacc
import concourse.bass as bass
import concourse.mybir as mybir
from concourse.tile import TileContext
from concourse.bass_utils import run_bass_kernel_spmd

F32 = mybir.dt.float32
BF16 = mybir.dt.bfloat16
AF = mybir.ActivationFunctionType
AX = mybir.AxisListType

B, S, D, DFF, K = 8, 512, 128, 256, 3
NOFF = 35  # 1 + 9 + 25 conv taps across the three inception branches
WCHUNK = 7  # offsets per weight DMA chunk
NWCH = 5    # 35 / 7

_last_exec_time_ns = None
_last_results = None


def _offsets():
    offs = []
    for ksz in (1, 3, 5):
        q = (ksz - 1) // 2
        for kh in range(ksz):
            for kw in range(ksz):
                offs.append((q, kh, kw))
    return offs


OFFS = _offsets()


def _row_chunks(p, n):
    """Split the p x n image (h-major) into row groups of <=512 columns."""
    ch = max(1, min(p, 512 // n))
    out = []
    h0 = 0
    while h0 < p:
        rows = min(ch, p - h0)
        out.append((h0, rows))
        h0 += rows
    return out


def _build_program(geoms):
    """geoms: [8][3] list of (p, n) per core per pass."""
    nc = bacc.Bacc()

    xb = nc.declare_dram_parameter("xb", [S, D], F32, isOutput=False)
    w1 = nc.declare_dram_parameter("w1", [D, NOFF * 2 * 128], BF16, isOutput=False)
    w2 = nc.declare_dram_parameter("w2", [D, NOFF * 2 * 128], BF16, isOutput=False)
    b1 = nc.declare_dram_parameter("b1", [DFF, 1], F32, isOutput=False)
    eb = nc.declare_dram_parameter("eb", [D, 1], F32, isOutput=False)
    enh = nc.declare_dram_parameter("enh", [D, D], F32, isOutput=False)
    ident = nc.declare_dram_parameter("ident", [128, 128], F32, isOutput=False)
    aw1 = nc.declare_dram_parameter("aw1", [1, K * 2 * K], F32, isOutput=False)
    aw2 = nc.declare_dram_parameter("aw2", [1, 2 * K * K], F32, isOutput=False)
    ab1 = nc.declare_dram_parameter("ab1", [1, 2 * K], F32, isOutput=False)
    ab2 = nc.declare_dram_parameter("ab2", [1, K], F32, isOutput=False)
    pwd = nc.declare_dram_parameter("pw", [1, K], F32, isOutput=False)
    out_d = nc.declare_dram_parameter("out", [S, D], F32, isOutput=True)

    with TileContext(nc) as tc:
        with (
            tc.tile_pool(name="persist", bufs=1) as pp,
            tc.tile_pool(name="work", bufs=1) as wp,
            tc.tile_pool(name="psum", bufs=1, space="PSUM") as qp,
        ):
            # ---- persistent loads (identical data on every core) ----
            identS = pp.tile([128, 128], F32, tag="identS", name="identS")
            nc.sync.dma_start(out=identS, in_=ident[:, :])

            xall = pp.tile([128, S], F32, tag="xall", name="xall")
            for c in range(4):
                nc.sync.dma_start(out=xall[:, c * 128:(c + 1) * 128],
                                  in_=xb[c * 128:(c + 1) * 128, :])

            w1c = []
            for ci in range(NWCH):
                t = pp.tile([128, WCHUNK * 2 * 128], BF16, tag=f"w1c{ci}",
                            name=f"w1c{ci}")
                nc.sync.dma_start(
                    out=t, in_=w1[:, ci * WCHUNK * 256:(ci + 1) * WCHUNK * 256])
                w1c.append(t)

            b1S = pp.tile([128, 2], F32, tag="b1S", name="b1S")
            nc.sync.dma_start(out=b1S[:, 0:1], in_=b1[0:128, :])
            nc.sync.dma_start(out=b1S[:, 1:2], in_=b1[128:256, :])
            ebS = pp.tile([128, 1], F32, tag="ebS", name="ebS")
            nc.sync.dma_start(out=ebS, in_=eb[:, :])
            enhS = pp.tile([128, 128], F32, tag="enhS", name="enhS")
            nc.sync.dma_start(out=enhS, in_=enh[:, :])
            aw1S = pp.tile([1, K * 2 * K], F32, tag="aw1S", name="aw1S")
            nc.sync.dma_start(out=aw1S, in_=aw1[:, :])
            aw2S = pp.tile([1, 2 * K * K], F32, tag="aw2S", name="aw2S")
            nc.sync.dma_start(out=aw2S, in_=aw2[:, :])
            ab1S = pp.tile([1, 2 * K], F32, tag="ab1S", name="ab1S")
            nc.sync.dma_start(out=ab1S, in_=ab1[:, :])
            ab2S = pp.tile([1, K], F32, tag="ab2S", name="ab2S")
            nc.sync.dma_start(out=ab2S, in_=ab2[:, :])
            pwS = pp.tile([1, K], F32, tag="pwS", name="pwS")
            nc.sync.dma_start(out=pwS, in_=pwd[:, :])

            w2c = []
            for ci in range(NWCH):
                t = pp.tile([128, WCHUNK * 2 * 128], BF16, tag=f"w2c{ci}",
                            name=f"w2c{ci}")
                nc.sync.dma_start(
                    out=t, in_=w2[:, ci * WCHUNK * 256:(ci + 1) * WCHUNK * 256])
                w2c.append(t)

            # constants
            onesM = pp.tile([128, 1], F32, tag="onesM", name="onesM")
            nc.vector.memset(onesM, 1.0 / 128.0)
            ones1_128 = pp.tile([1, 128], F32, tag="ones1_128", name="ones1_128")
            nc.vector.memset(ones1_128, 1.0)

            # ---- transpose x: [S, D] -> xT [D, S] ----
            xT = pp.tile([128, S], F32, tag="xT", name="xT")
            for c in range(4):
                pt = qp.tile([128, 512], F32, tag="ps_small", bufs=2,
                             name=f"ptx{c}")
                nc.tensor.transpose(pt[:, 0:128],
                                    xall[:, c * 128:(c + 1) * 128], identS)
                nc.vector.tensor_copy(out=xT[:, c * 128:(c + 1) * 128],
                                      in_=pt[:, 0:128])

            def w1_lhsT(o, mc):
                ci, oi = divmod(o, WCHUNK)
                base = (oi * 2 + mc) * 128
                return w1c[ci][:, base:base + 128]

            def w2_lhsT(o, kc):
                ci, oi = divmod(o, WCHUNK)
                base = (oi * 2 + kc) * 128
                return w2c[ci][:, base:base + 128]

            def _transpose_po(core, k, po_k, poT_k):
                for c in range(4):
                    pt = qp.tile([128, 512], F32, tag="ps_small", bufs=2,
                                 name=f"ptp{k}{c}_{core}")
                    nc.tensor.transpose(pt[:, 0:128],
                                        po_k[:, c * 128:(c + 1) * 128],
                                        identS)
                    nc.vector.tensor_copy(
                        out=poT_k[:, c * 128:(c + 1) * 128], in_=pt[:, 0:128])

            def _fill_canvas(core, k, cv):
                p, n = geoms[core][k]
                nfull = S // p
                r = S - nfull * p
                src = xT[:, 0:nfull * p].rearrange("d (w h) -> d h w", h=p)
                nc.vector.tensor_copy(out=cv[:, 2:2 + p, 2:2 + nfull],
                                      in_=src)
                if r > 0:
                    tsrc = xT[:, nfull * p:S].rearrange("d (h w) -> d h w",
                                                        w=1)
                    nc.vector.tensor_copy(
                        out=cv[:, 2:2 + r, 2 + nfull:3 + nfull], in_=tsrc)

            def build_arm(core):
                # allocate per-pass tiles; memset canvases early so DVE works
                # while the weight DMAs stream in
                cvs, hcs = [], []
                for k in range(K):
                    p, n = geoms[core][k]
                    Hp, Wp = p + 4, n + 4
                    canvas = wp.tile([128, Hp * Wp], BF16, tag=f"canvas{k}",
                                     name=f"canvas{k}_{core}")
                    nc.gpsimd.memset(canvas, 0.0)
                    cvs.append(canvas.rearrange("d (h w) -> d h w", w=Wp))
                    if k == 0:
                        _fill_canvas(core, 0, cvs[0])
                    hk = []
                    for mc in range(2):
                        hc = wp.tile([128, Hp * Wp], BF16, tag=f"hc{k}_{mc}",
                                     name=f"hc{k}_{mc}_{core}")
                        nc.gpsimd.memset(hc, 0.0)
                        hk.append(hc.rearrange("d (h w) -> d h w", w=Wp))
                    hcs.append(hk)

                po, poT = [], []
                c2s, ps3s = {}, {}
                polastT = wp.tile([128, K], F32, tag="polast",
                                  name=f"polast_{core}")

                def finish_pass(k):
                    p, n = geoms[core][k]
                    L = p * n
                    nfull = S // p
                    r = S - nfull * p
                    c2 = c2s[k]
                    ps3 = qp.tile([128, 512], F32, tag="ps_small", bufs=2,
                                  name=f"ps3_{core}_{k}")
                    c2v = c2[:, 0:L].rearrange("d (h w) -> d w h", w=n)
                    nc.tensor.matmul(ps3[:, 0:nfull * p], enhS,
                                     c2v[:, 0:nfull, :],
                                     start=True, stop=(r == 0))
                    if r > 0:
                        nc.tensor.matmul(ps3[:, nfull * p:S], enhS,
                                         c2v[:, nfull:nfull + 1, 0:r],
                                         start=False, stop=True)
                    po_k = wp.tile([128, S], F32, tag=f"po{k}",
                                   name=f"po{k}_{core}")
                    nc.scalar.activation(out=po_k, in_=ps3[:, 0:S],
                                         func=AF.Gelu, bias=ebS[:, 0:1])
                    po.append(po_k)
                    nc.vector.tensor_copy(out=polastT[:, k:k + 1],
                                          in_=po_k[:, S - 1:S])
                    poT_k = wp.tile([128, S], F32, tag=f"poT{k}",
                                    name=f"poT{k}_{core}")
                    if k < 2:
                        _transpose_po(core, k, po_k, poT_k)
                    poT.append(poT_k)

                for k in range(K):
                    p, n = geoms[core][k]
                    L = p * n
                    nfull = S // p
                    r = S - nfull * p
                    chunks = _row_chunks(p, n)
                    assert len(chunks) <= 2, (p, n, chunks)
                    cv, hcv = cvs[k], hcs[k]

                    if k > 0:
                        _fill_canvas(core, k, cv)

                    # conv1: accumulate 35 taps per (Cout chunk, row chunk)
                    ps1 = [qp.tile([128, 1024], F32, tag=f"c1p{mc}",
                                   name=f"c1p{mc}_{core}_{k}")
                           for mc in range(2)]
                    for mc in range(2):
                        for o, (q, kh, kw) in enumerate(OFFS):
                            for j, (h0, rows) in enumerate(chunks):
                                rhs = cv[:, 2 - q + kh + h0:
                                         2 - q + kh + h0 + rows,
                                         2 - q + kw:2 - q + kw + n]
                                nc.tensor.matmul(
                                    ps1[mc][:, j * 512:j * 512 + rows * n],
                                    w1_lhsT(o, mc), rhs,
                                    start=(o == 0), stop=(o == NOFF - 1))
                        # gelu(conv1 + b1) written into padded h-canvas
                        for j, (h0, rows) in enumerate(chunks):
                            nc.scalar.activation(
                                out=hcv[mc][:, 2 + h0:2 + h0 + rows, 2:2 + n],
                                in_=ps1[mc][:, j * 512:j * 512 + rows * n],
                                func=AF.Gelu, bias=b1S[:, mc:mc + 1])

                    if k > 0:
                        finish_pass(k - 1)

                    # conv2: contract over 256 channels (2 K-chunks) x 35 taps
                    ps2 = qp.tile([128, 1024], F32, tag="c2p",
                                  name=f"c2p_{core}_{k}")
                    for kc in range(2):
                        for o, (q, kh, kw) in enumerate(OFFS):
                            for j, (h0, rows) in enumerate(chunks):
                                rhs = hcv[kc][:, 2 - q + kh + h0:
                                              2 - q + kh + h0 + rows,
                                              2 - q + kw:2 - q + kw + n]
                                nc.tensor.matmul(
                                    ps2[:, j * 512:j * 512 + rows * n],
                                    w2_lhsT(o, kc), rhs,
                                    start=(kc == 0 and o == 0),
                                    stop=(kc == 1 and o == NOFF - 1))

                    # conv2 out back to SBUF, contiguous h-major
                    c2 = wp.tile([128, 1024], F32, tag="c2", bufs=2,
                                 name=f"c2_{core}_{k}")
                    for j, (h0, rows) in enumerate(chunks):
                        nc.vector.tensor_copy(
                            out=c2[:, h0 * n:h0 * n + rows * n],
                            in_=ps2[:, j * 512:j * 512 + rows * n])
                    c2s[k] = c2

                finish_pass(2)

                # ---- combination weights (free-dim chain on partition 0) ----
                lsfP = qp.tile([1, K], F32, tag="ps_small", bufs=2,
                               name=f"lsfP_{core}")
                nc.tensor.matmul(lsfP, onesM, polastT, start=True, stop=True)
                lsfS = wp.tile([1, K], F32, tag="lsfS", name=f"lsfS_{core}")
                nc.vector.tensor_copy(out=lsfS, in_=lsfP)

                # attn MLP on partition 0; aw1/aw2 stored column-major
                t6 = wp.tile([1, 2 * K], F32, tag="t6", name=f"t6_{core}")
                tt6 = wp.tile([1, 2 * K], F32, tag="tt6", name=f"tt6_{core}")
                aw1r = aw1S.rearrange("p (j i) -> p i j", i=K)
                nc.vector.tensor_scalar_mul(t6, aw1r[:, 0, :], lsfS[:, 0:1])
                nc.vector.tensor_scalar_mul(tt6, aw1r[:, 1, :], lsfS[:, 1:2])
                nc.vector.tensor_add(t6, t6, tt6)
                nc.vector.tensor_scalar_mul(tt6, aw1r[:, 2, :], lsfS[:, 2:3])
                nc.vector.tensor_add(t6, t6, tt6)
                nc.vector.tensor_add(t6, t6, ab1S)
                nc.vector.tensor_scalar_max(t6, t6, 0.0)

                a3 = wp.tile([1, K], F32, tag="a3", name=f"a3_{core}")
                tt3 = wp.tile([1, K], F32, tag="tt3", name=f"tt3_{core}")
                aw2r = aw2S.rearrange("p (j i) -> p i j", i=2 * K)
                nc.vector.tensor_scalar_mul(a3, aw2r[:, 0, :], t6[:, 0:1])
                for i in range(1, 6):
                    nc.vector.tensor_scalar_mul(tt3, aw2r[:, i, :],
                                                t6[:, i:i + 1])
                    nc.vector.tensor_add(a3, a3, tt3)
                nc.vector.tensor_add(a3, a3, ab2S)

                e1 = wp.tile([1, K], F32, tag="e1", name=f"e1_{core}")
                nc.scalar.activation(out=e1, in_=a3, func=AF.Exp)
                s1 = wp.tile([1, 1], F32, tag="s1", name=f"s1_{core}")
                nc.vector.reduce_sum(out=s1, in_=e1, axis=AX.X)
                r1 = wp.tile([1, 1], F32, tag="r1", name=f"r1_{core}")
                nc.vector.reciprocal(r1, s1)
                z = wp.tile([1, K], F32, tag="z", name=f"z_{core}")
                nc.vector.tensor_scalar_mul(z, e1, r1[:, 0:1])
                nc.vector.tensor_mul(z, z, pwS)
                e2 = wp.tile([1, K], F32, tag="e2", name=f"e2_{core}")
                nc.scalar.activation(out=e2, in_=z, func=AF.Exp)
                s2 = wp.tile([1, 1], F32, tag="s2", name=f"s2_{core}")
                nc.vector.reduce_sum(out=s2, in_=e2, axis=AX.X)
                r2 = wp.tile([1, 1], F32, tag="r2", name=f"r2_{core}")
                nc.vector.reciprocal(r2, s2)
                cwrow = wp.tile([1, K], F32, tag="cwrow", name=f"cwrow_{core}")
                nc.vector.tensor_scalar_mul(cwrow, e2, r2[:, 0:1])

                cwbP = qp.tile([128, K], F32, tag="ps_small", bufs=2,
                               name=f"cwbP_{core}")
                nc.tensor.matmul(cwbP, ones1_128, cwrow, start=True, stop=True)
                cwS = wp.tile([128, K], F32, tag="cwS", name=f"cwS_{core}")
                nc.vector.tensor_copy(out=cwS, in_=cwbP)

                _transpose_po(core, 2, po[2], poT[2])
                # ---- weighted combine + residual, already in [s, d] ----
                accT = wp.tile([128, S], F32, tag="accT", name=f"accT_{core}")
                q1T = wp.tile([128, S], F32, tag="q1T", name=f"q1T_{core}")
                q2T = wp.tile([128, S], F32, tag="q2T", name=f"q2T_{core}")
                nc.scalar.activation(out=q1T, in_=poT[1], func=AF.Copy,
                                     scale=cwS[:, 1:2])
                nc.scalar.activation(out=q2T, in_=poT[2], func=AF.Copy,
                                     scale=cwS[:, 2:3])
                nc.vector.tensor_scalar_mul(accT, poT[0], cwS[:, 0:1])
                nc.vector.tensor_add(accT, accT, xall)
                nc.vector.tensor_add(accT, accT, q1T)
                nc.vector.tensor_add(accT, accT, q2T)

                nc.sync.dma_start(
                    out=out_d.rearrange("(c sp) d -> sp c d", sp=128),
                    in_=accT.rearrange("sp (c d) -> sp c d", d=128))

            pid = nc.partition_id(engines=mybir.ALL_ENGINES)
            for core in tc.Switch(pid, 8):
                build_arm(core)

    nc.finalize()
    return nc


def _prep_inputs(x, conv1_w, conv1_b, conv2_w, conv2_b, attn_w1, attn_b1,
                 attn_w2, attn_b2, enh_w, enh_b):
    x = np.asarray(x, dtype=np.float32)
    conv1_w = [np.asarray(w, dtype=np.float32) for w in conv1_w]
    conv1_b = [np.asarray(w, dtype=np.float32) for w in conv1_b]
    conv2_w = [np.asarray(w, dtype=np.float32) for w in conv2_w]
    conv2_b = [np.asarray(w, dtype=np.float32) for w in conv2_b]
    enh_w = np.asarray(enh_w, dtype=np.float32)
    enh_b = np.asarray(enh_b, dtype=np.float32)

    # FFT period detection on host (reference does the same via .item())
    xf = np.abs(np.fft.rfft(x, axis=1)).mean(-1).astype(np.float32)
    xf[:, 0] = 0.0
    top = np.argsort(-xf, axis=1, kind="stable")[:, :K]
    geoms, pws = [], []
    for b in range(B):
        g = []
        for k in range(K):
            f = int(top[b, k])
            p = S if f == 0 else max(1, S // f)
            pad = (-S) % p
            g.append((p, (S + pad) // p))
        geoms.append(g)
        wv = xf[b, top[b]].astype(np.float64)
        ev = np.exp(wv - wv.max())
        pws.append((ev / ev.sum()).astype(np.float32).reshape(1, K))

    # conv weights -> per-tap transposed matmul operands, pre-scaled by 1/3
    w1A = np.zeros((128, NOFF, 2, 128), dtype=np.float32)
    w2A = np.zeros((128, NOFF, 2, 128), dtype=np.float32)
    o = 0
    for br, ksz in enumerate((1, 3, 5)):
        for kh in range(ksz):
            for kw in range(ksz):
                wt1 = conv1_w[br][:, :, kh, kw] / 3.0   # [256(out), 128(in)]
                w1A[:, o, 0, :] = wt1[0:128, :].T
                w1A[:, o, 1, :] = wt1[128:256, :].T
                wt2 = conv2_w[br][:, :, kh, kw] / 3.0   # [128(out), 256(in)]
                w2A[:, o, 0, :] = wt2[:, 0:128].T
                w2A[:, o, 1, :] = wt2[:, 128:256].T
                o += 1
    import ml_dtypes
    w1A = w1A.reshape(128, NOFF * 2 * 128).astype(ml_dtypes.bfloat16)
    w2A = w2A.reshape(128, NOFF * 2 * 128).astype(ml_dtypes.bfloat16)
    b1m = (sum(conv1_b) / 3.0).reshape(DFF, 1).astype(np.float32)
    b2m = (sum(conv2_b) / 3.0).astype(np.float32)
    ebv = (b2m @ enh_w + enh_b).reshape(D, 1).astype(np.float32)

    shared = dict(
        w1=w1A, w2=w2A, b1=b1m, eb=ebv,
        enh=np.ascontiguousarray(enh_w),
        ident=np.eye(128, dtype=np.float32),
        aw1=np.ascontiguousarray(np.asarray(attn_w1, dtype=np.float32).T
                                 ).reshape(1, K * 2 * K),
        aw2=np.ascontiguousarray(np.asarray(attn_w2, dtype=np.float32).T
                                 ).reshape(1, 2 * K * K),
        ab1=np.asarray(attn_b1, dtype=np.float32).reshape(1, 2 * K),
        ab2=np.asarray(attn_b2, dtype=np.float32).reshape(1, K),
    )
    in_maps = []
    for b in range(B):
        m = dict(shared)
        m["xb"] = np.ascontiguousarray(x[b])
        m["pw"] = pws[b]
        in_maps.append(m)
    return geoms, in_maps


def _ensure_axon_hooks():
    """bass_utils' trace path imports antenv.axon_hooks unconditionally;
    the container's antenv lacks it. Provide it, registering the real
    ctypes NTFF hook when available so tracing works."""
    try:
        import antenv.axon_hooks  # noqa: F401
        return
    except Exception:
        pass
    import sys
    import types

    import antenv

    m = types.ModuleType("antenv.axon_hooks")
    m._h = None
    m.set_axon_ntff_profile_hook = lambda h: setattr(m, "_h", h)
    m.get_axon_ntff_profile_hook = lambda: m._h
    sys.modules["antenv.axon_hooks"] = m
    antenv.axon_hooks = m
    try:
        from trn_agent_boot.trn_boot import _ntff_profile_via_ctypes

        m._h = _ntff_profile_via_ctypes("/opt/axon/libaxon_pjrt.so")
    except Exception:
        pass


def kernel(**inputs):
    global _last_exec_time_ns, _last_results
    _ensure_axon_hooks()
    geoms, in_maps = _prep_inputs(**inputs)
    nc = _build_program(geoms)
    res = run_bass_kernel_spmd(nc, in_maps, core_ids=list(range(8)))
    _last_results = res
    _last_exec_time_ns = res.exec_time_ns
    out = np.stack([res.results[i]["out"] for i in range(8)], axis=0)
    return out.astype(np.float32)


# revision 10
# speedup vs baseline: 1.3662x; 1.0132x over previous
"""AdaptivePeriodicLayer Trainium2 kernel.

Strategy: data-parallel over batch (8 samples -> 8 NeuronCores), one SPMD
program with an 8-way tc.Switch on partition id. Each arm is specialized at
build time to that sample's 3 FFT-detected period geometries (computed on
host, like the reference's .item() round-trip). Convs are computed as
per-kernel-offset matmuls accumulated in PSUM over a zero-padded 2D canvas
held in SBUF; conv operands are bf16 (fp32 PSUM accumulation), everything
else fp32.
"""

import numpy as np

import concourse.bacc as bacc
import concourse.bass as bass
import concourse.mybir as mybir
from concourse.tile import TileContext
from concourse.bass_utils import run_bass_kernel_spmd

F32 = mybir.dt.float32
BF16 = mybir.dt.bfloat16
AF = mybir.ActivationFunctionType
AX = mybir.AxisListType

B, S, D, DFF, K = 8, 512, 128, 256, 3
NOFF = 35  # 1 + 9 + 25 conv taps across the three inception branches
WCHUNK = 7  # offsets per weight DMA chunk
NWCH = 5    # 35 / 7

_last_exec_time_ns = None
_last_results = None


def _offsets():
    offs = []
    for ksz in (1, 3, 5):
        q = (ksz - 1) // 2
        for kh in range(ksz):
            for kw in range(ksz):
                offs.append((q, kh, kw))
    return offs


OFFS = _offsets()


def _row_chunks(p, n):
    """Split the p x n image (h-major) into row groups of <=512 columns."""
    ch = max(1, min(p, 512 // n))
    out = []
    h0 = 0
    while h0 < p:
        rows = min(ch, p - h0)
        out.append((h0, rows))
        h0 += rows
    return out


def _build_program(geoms):
    """geoms: [8][3] list of (p, n) per core per pass."""
    nc = bacc.Bacc()

    xb = nc.declare_dram_parameter("xb", [S, D], F32, isOutput=False)
    w1 = nc.declare_dram_parameter("w1", [D, NOFF * 2 * 128], BF16, isOutput=False)
    w2 = nc.declare_dram_parameter("w2", [D, NOFF * 2 * 128], BF16, isOutput=False)
    b1 = nc.declare_dram_parameter("b1", [DFF, 1], F32, isOutput=False)
    eb = nc.declare_dram_parameter("eb", [D, 1], F32, isOutput=False)
    enh = nc.declare_dram_parameter("enh", [D, D], F32, isOutput=False)
    ident = nc.declare_dram_parameter("ident", [128, 128], F32, isOutput=False)
    aw1 = nc.declare_dram_parameter("aw1", [1, K * 2 * K], F32, isOutput=False)
    aw2 = nc.declare_dram_parameter("aw2", [1, 2 * K * K], F32, isOutput=False)
    ab1 = nc.declare_dram_parameter("ab1", [1, 2 * K], F32, isOutput=False)
    ab2 = nc.declare_dram_parameter("ab2", [1, K], F32, isOutput=False)
    pwd = nc.declare_dram_parameter("pw", [1, K], F32, isOutput=False)
    out_d = nc.declare_dram_parameter("out", [S, D], F32, isOutput=True)

    with TileContext(nc) as tc:
        with (
            tc.tile_pool(name="persist", bufs=1) as pp,
            tc.tile_pool(name="work", bufs=1) as wp,
            tc.tile_pool(name="psum", bufs=1, space="PSUM") as qp,
        ):
            # partition id + arm-prefetch hint as early as possible so the
            # IRAM block of this core's Switch arm is resident at dispatch
            pid = nc.partition_id(engines=mybir.ALL_ENGINES)
            shint = tc.switch_hint({e: pid for e in mybir.ALL_ENGINES}, 8,
                                   "arms")

            # ---- persistent loads (identical data on every core) ----
            identS = pp.tile([128, 128], F32, tag="identS", name="identS")
            nc.sync.dma_start(out=identS, in_=ident[:, :])

            xall = pp.tile([128, S], F32, tag="xall", name="xall")
            for c in range(4):
                nc.sync.dma_start(out=xall[:, c * 128:(c + 1) * 128],
                                  in_=xb[c * 128:(c + 1) * 128, :])

            w1c = []
            for ci in range(NWCH):
                t = pp.tile([128, WCHUNK * 2 * 128], BF16, tag=f"w1c{ci}",
                            name=f"w1c{ci}")
                nc.sync.dma_start(
                    out=t, in_=w1[:, ci * WCHUNK * 256:(ci + 1) * WCHUNK * 256])
                w1c.append(t)

            b1S = pp.tile([128, 2], F32, tag="b1S", name="b1S")
            nc.sync.dma_start(out=b1S[:, 0:1], in_=b1[0:128, :])
            nc.sync.dma_start(out=b1S[:, 1:2], in_=b1[128:256, :])
            ebS = pp.tile([128, 1], F32, tag="ebS", name="ebS")
            nc.sync.dma_start(out=ebS, in_=eb[:, :])
            enhS = pp.tile([128, 128], F32, tag="enhS", name="enhS")
            nc.sync.dma_start(out=enhS, in_=enh[:, :])
            aw1S = pp.tile([1, K * 2 * K], F32, tag="aw1S", name="aw1S")
            nc.sync.dma_start(out=aw1S, in_=aw1[:, :])
            aw2S = pp.tile([1, 2 * K * K], F32, tag="aw2S", name="aw2S")
            nc.sync.dma_start(out=aw2S, in_=aw2[:, :])
            ab1S = pp.tile([1, 2 * K], F32, tag="ab1S", name="ab1S")
            nc.sync.dma_start(out=ab1S, in_=ab1[:, :])
            ab2S = pp.tile([1, K], F32, tag="ab2S", name="ab2S")
            nc.sync.dma_start(out=ab2S, in_=ab2[:, :])
            pwS = pp.tile([1, K], F32, tag="pwS", name="pwS")
            nc.sync.dma_start(out=pwS, in_=pwd[:, :])

            w2c = []
            for ci in range(NWCH):
                t = pp.tile([128, WCHUNK * 2 * 128], BF16, tag=f"w2c{ci}",
                            name=f"w2c{ci}")
                nc.sync.dma_start(
                    out=t, in_=w2[:, ci * WCHUNK * 256:(ci + 1) * WCHUNK * 256])
                w2c.append(t)

            # constants
            onesM = pp.tile([128, 1], F32, tag="onesM", name="onesM")
            nc.vector.memset(onesM, 1.0 / 128.0)
            ones1_128 = pp.tile([1, 128], F32, tag="ones1_128", name="ones1_128")
            nc.vector.memset(ones1_128, 1.0)

            # ---- transpose x: [S, D] -> xT [D, S] ----
            xT = pp.tile([128, S], F32, tag="xT", name="xT")
            for c in range(4):
                pt = qp.tile([128, 512], F32, tag="ps_small", bufs=2,
                             name=f"ptx{c}")
                nc.tensor.transpose(pt[:, 0:128],
                                    xall[:, c * 128:(c + 1) * 128], identS)
                nc.vector.tensor_copy(out=xT[:, c * 128:(c + 1) * 128],
                                      in_=pt[:, 0:128])

            def w1_lhsT(o, mc):
                ci, oi = divmod(o, WCHUNK)
                base = (oi * 2 + mc) * 128
                return w1c[ci][:, base:base + 128]

            def w2_lhsT(o, kc):
                ci, oi = divmod(o, WCHUNK)
                base = (oi * 2 + kc) * 128
                return w2c[ci][:, base:base + 128]

            def _transpose_po(core, k, po_k, poT_k):
                for c in range(4):
                    pt = qp.tile([128, 512], F32, tag="ps_small", bufs=2,
                                 name=f"ptp{k}{c}_{core}")
                    nc.tensor.transpose(pt[:, 0:128],
                                        po_k[:, c * 128:(c + 1) * 128],
                                        identS)
                    nc.vector.tensor_copy(
                        out=poT_k[:, c * 128:(c + 1) * 128], in_=pt[:, 0:128])

            def _fill_canvas(core, k, cv):
                p, n = geoms[core][k]
                nfull = S // p
                r = S - nfull * p
                src = xT[:, 0:nfull * p].rearrange("d (w h) -> d h w", h=p)
                nc.vector.tensor_copy(out=cv[:, 2:2 + p, 2:2 + nfull],
                                      in_=src)
                if r > 0:
                    tsrc = xT[:, nfull * p:S].rearrange("d (h w) -> d h w",
                                                        w=1)
                    nc.vector.tensor_copy(
                        out=cv[:, 2:2 + r, 2 + nfull:3 + nfull], in_=tsrc)

            def build_arm(core):
                # allocate per-pass tiles; memset canvases early so DVE works
                # while the weight DMAs stream in
                cvs, hcs = [], []
                for k in range(K):
                    p, n = geoms[core][k]
                    Hp, Wp = p + 4, n + 4
                    canvas = wp.tile([128, Hp * Wp], BF16, tag=f"canvas{k}",
                                     name=f"canvas{k}_{core}")
                    nc.gpsimd.memset(canvas, 0.0)
                    cvs.append(canvas.rearrange("d (h w) -> d h w", w=Wp))
                    if k == 0:
                        _fill_canvas(core, 0, cvs[0])
                    hk = []
                    for mc in range(2):
                        hc = wp.tile([128, Hp * Wp], BF16, tag=f"hc{k}_{mc}",
                                     name=f"hc{k}_{mc}_{core}")
                        nc.gpsimd.memset(hc, 0.0)
                        hk.append(hc.rearrange("d (h w) -> d h w", w=Wp))
                    hcs.append(hk)

                po, poT = [], []
                c2s, ps3s = {}, {}
                polastT = wp.tile([128, K], F32, tag="polast",
                                  name=f"polast_{core}")

                def finish_pass(k):
                    p, n = geoms[core][k]
                    L = p * n
                    nfull = S // p
                    r = S - nfull * p
                    c2 = c2s[k]
                    ps3 = qp.tile([128, 512], F32, tag="ps_small", bufs=2,
                                  name=f"ps3_{core}_{k}")
                    c2v = c2[:, 0:L].rearrange("d (h w) -> d w h", w=n)
                    nc.tensor.matmul(ps3[:, 0:nfull * p], enhS,
                                     c2v[:, 0:nfull, :],
                                     start=True, stop=(r == 0))
                    if r > 0:
                        nc.tensor.matmul(ps3[:, nfull * p:S], enhS,
                                         c2v[:, nfull:nfull + 1, 0:r],
                                         start=False, stop=True)
                    po_k = wp.tile([128, S], F32, tag=f"po{k}",
                                   name=f"po{k}_{core}")
                    nc.scalar.activation(out=po_k, in_=ps3[:, 0:S],
                                         func=AF.Gelu, bias=ebS[:, 0:1])
                    po.append(po_k)
                    nc.vector.tensor_copy(out=polastT[:, k:k + 1],
                                          in_=po_k[:, S - 1:S])
                    poT_k = wp.tile([128, S], F32, tag=f"poT{k}",
                                    name=f"poT{k}_{core}")
                    if k < 2:
                        _transpose_po(core, k, po_k, poT_k)
                    poT.append(poT_k)

                for k in range(K):
                    p, n = geoms[core][k]
                    L = p * n
                    nfull = S // p
                    r = S - nfull * p
                    chunks = _row_chunks(p, n)
                    assert len(chunks) <= 2, (p, n, chunks)
                    cv, hcv = cvs[k], hcs[k]

                    if k > 0:
                        _fill_canvas(core, k, cv)

                    # conv1: accumulate 35 taps per (Cout chunk, row chunk)
                    ps1 = [qp.tile([128, 1024], F32, tag=f"c1p{mc}",
                                   name=f"c1p{mc}_{core}_{k}")
                           for mc in range(2)]
                    for mc in range(2):
                        for o, (q, kh, kw) in enumerate(OFFS):
                            for j, (h0, rows) in enumerate(chunks):
                                rhs = cv[:, 2 - q + kh + h0:
                                         2 - q + kh + h0 + rows,
                                         2 - q + kw:2 - q + kw + n]
                                nc.tensor.matmul(
                                    ps1[mc][:, j * 512:j * 512 + rows * n],
                                    w1_lhsT(o, mc), rhs,
                                    start=(o == 0), stop=(o == NOFF - 1))
                        # gelu(conv1 + b1) written into padded h-canvas
                        for j, (h0, rows) in enumerate(chunks):
                            nc.scalar.activation(
                                out=hcv[mc][:, 2 + h0:2 + h0 + rows, 2:2 + n],
                                in_=ps1[mc][:, j * 512:j * 512 + rows * n],
                                func=AF.Gelu, bias=b1S[:, mc:mc + 1])

                    if k > 0:
                        finish_pass(k - 1)

                    # conv2: contract over 256 channels (2 K-chunks) x 35 taps
                    ps2 = qp.tile([128, 1024], F32, tag="c2p",
                                  name=f"c2p_{core}_{k}")
                    for kc in range(2):
                        for o, (q, kh, kw) in enumerate(OFFS):
                            for j, (h0, rows) in enumerate(chunks):
                                rhs = hcv[kc][:, 2 - q + kh + h0:
                                              2 - q + kh + h0 + rows,
                                              2 - q + kw:2 - q + kw + n]
                                nc.tensor.matmul(
                                    ps2[:, j * 512:j * 512 + rows * n],
                                    w2_lhsT(o, kc), rhs,
                                    start=(kc == 0 and o == 0),
                                    stop=(kc == 1 and o == NOFF - 1))

                    # conv2 out back to SBUF, contiguous h-major
                    c2 = wp.tile([128, 1024], F32, tag="c2", bufs=2,
                                 name=f"c2_{core}_{k}")
                    for j, (h0, rows) in enumerate(chunks):
                        nc.vector.tensor_copy(
                            out=c2[:, h0 * n:h0 * n + rows * n],
                            in_=ps2[:, j * 512:j * 512 + rows * n])
                    c2s[k] = c2

                finish_pass(2)

                # ---- combination weights (free-dim chain on partition 0) ----
                lsfP = qp.tile([1, K], F32, tag="ps_small", bufs=2,
                               name=f"lsfP_{core}")
                nc.tensor.matmul(lsfP, onesM, polastT, start=True, stop=True)
                lsfS = wp.tile([1, K], F32, tag="lsfS", name=f"lsfS_{core}")
                nc.vector.tensor_copy(out=lsfS, in_=lsfP)

                # attn MLP on partition 0; aw1/aw2 stored column-major
                t6 = wp.tile([1, 2 * K], F32, tag="t6", name=f"t6_{core}")
                tt6 = wp.tile([1, 2 * K], F32, tag="tt6", name=f"tt6_{core}")
                aw1r = aw1S.rearrange("p (j i) -> p i j", i=K)
                nc.vector.tensor_scalar_mul(t6, aw1r[:, 0, :], lsfS[:, 0:1])
                nc.vector.tensor_scalar_mul(tt6, aw1r[:, 1, :], lsfS[:, 1:2])
                nc.vector.tensor_add(t6, t6, tt6)
                nc.vector.tensor_scalar_mul(tt6, aw1r[:, 2, :], lsfS[:, 2:3])
                nc.vector.tensor_add(t6, t6, tt6)
                nc.vector.tensor_add(t6, t6, ab1S)
                nc.vector.tensor_scalar_max(t6, t6, 0.0)

                a3 = wp.tile([1, K], F32, tag="a3", name=f"a3_{core}")
                tt3 = wp.tile([1, K], F32, tag="tt3", name=f"tt3_{core}")
                aw2r = aw2S.rearrange("p (j i) -> p i j", i=2 * K)
                nc.vector.tensor_scalar_mul(a3, aw2r[:, 0, :], t6[:, 0:1])
                for i in range(1, 6):
                    nc.vector.tensor_scalar_mul(tt3, aw2r[:, i, :],
                                                t6[:, i:i + 1])
                    nc.vector.tensor_add(a3, a3, tt3)
                nc.vector.tensor_add(a3, a3, ab2S)

                e1 = wp.tile([1, K], F32, tag="e1", name=f"e1_{core}")
                nc.scalar.activation(out=e1, in_=a3, func=AF.Exp)
                s1 = wp.tile([1, 1], F32, tag="s1", name=f"s1_{core}")
                nc.vector.reduce_sum(out=s1, in_=e1, axis=AX.X)
                r1 = wp.tile([1, 1], F32, tag="r1", name=f"r1_{core}")
                nc.vector.reciprocal(r1, s1)
                z = wp.tile([1, K], F32, tag="z", name=f"z_{core}")
                nc.vector.tensor_scalar_mul(z, e1, r1[:, 0:1])
                nc.vector.tensor_mul(z, z, pwS)
                e2 = wp.tile([1, K], F32, tag="e2", name=f"e2_{core}")
                nc.scalar.activation(out=e2, in_=z, func=AF.Exp)
                s2 = wp.tile([1, 1], F32, tag="s2", name=f"s2_{core}")
                nc.vector.reduce_sum(out=s2, in_=e2, axis=AX.X)
                r2 = wp.tile([1, 1], F32, tag="r2", name=f"r2_{core}")
                nc.vector.reciprocal(r2, s2)
                cwrow = wp.tile([1, K], F32, tag="cwrow", name=f"cwrow_{core}")
                nc.vector.tensor_scalar_mul(cwrow, e2, r2[:, 0:1])

                cwbP = qp.tile([128, K], F32, tag="ps_small", bufs=2,
                               name=f"cwbP_{core}")
                nc.tensor.matmul(cwbP, ones1_128, cwrow, start=True, stop=True)
                cwS = wp.tile([128, K], F32, tag="cwS", name=f"cwS_{core}")
                nc.vector.tensor_copy(out=cwS, in_=cwbP)

                _transpose_po(core, 2, po[2], poT[2])
                # ---- weighted combine + residual, already in [s, d] ----
                accT = wp.tile([128, S], F32, tag="accT", name=f"accT_{core}")
                q1T = wp.tile([128, S], F32, tag="q1T", name=f"q1T_{core}")
                q2T = wp.tile([128, S], F32, tag="q2T", name=f"q2T_{core}")
                nc.scalar.activation(out=q1T, in_=poT[1], func=AF.Copy,
                                     scale=cwS[:, 1:2])
                nc.scalar.activation(out=q2T, in_=poT[2], func=AF.Copy,
                                     scale=cwS[:, 2:3])
                nc.vector.tensor_scalar_mul(accT, poT[0], cwS[:, 0:1])
                nc.vector.tensor_add(accT, accT, xall)
                nc.vector.tensor_add(accT, accT, q1T)
                nc.vector.tensor_add(accT, accT, q2T)

                nc.sync.dma_start(
                    out=out_d.rearrange("(c sp) d -> sp c d", sp=128),
                    in_=accT.rearrange("sp (c d) -> sp c d", d=128))

            for core in tc.Switch(pid, 8, hint=shint):
                build_arm(core)

    nc.finalize()
    return nc


def _prep_inputs(x, conv1_w, conv1_b, conv2_w, conv2_b, attn_w1, attn_b1,
                 attn_w2, attn_b2, enh_w, enh_b):
    x = np.asarray(x, dtype=np.float32)
    conv1_w = [np.asarray(w, dtype=np.float32) for w in conv1_w]
    conv1_b = [np.asarray(w, dtype=np.float32) for w in conv1_b]
    conv2_w = [np.asarray(w, dtype=np.float32) for w in conv2_w]
    conv2_b = [np.asarray(w, dtype=np.float32) for w in conv2_b]
    enh_w = np.asarray(enh_w, dtype=np.float32)
    enh_b = np.asarray(enh_b, dtype=np.float32)

    # FFT period detection on host (reference does the same via .item())
    xf = np.abs(np.fft.rfft(x, axis=1)).mean(-1).astype(np.float32)
    xf[:, 0] = 0.0
    top = np.argsort(-xf, axis=1, kind="stable")[:, :K]
    geoms, pws = [], []
    for b in range(B):
        g = []
        for k in range(K):
            f = int(top[b, k])
            p = S if f == 0 else max(1, S // f)
            pad = (-S) % p
            g.append((p, (S + pad) // p))
        geoms.append(g)
        wv = xf[b, top[b]].astype(np.float64)
        ev = np.exp(wv - wv.max())
        pws.append((ev / ev.sum()).astype(np.float32).reshape(1, K))

    # conv weights -> per-tap transposed matmul operands, pre-scaled by 1/3
    w1A = np.zeros((128, NOFF, 2, 128), dtype=np.float32)
    w2A = np.zeros((128, NOFF, 2, 128), dtype=np.float32)
    o = 0
    for br, ksz in enumerate((1, 3, 5)):
        for kh in range(ksz):
            for kw in range(ksz):
                wt1 = conv1_w[br][:, :, kh, kw] / 3.0   # [256(out), 128(in)]
                w1A[:, o, 0, :] = wt1[0:128, :].T
                w1A[:, o, 1, :] = wt1[128:256, :].T
                wt2 = conv2_w[br][:, :, kh, kw] / 3.0   # [128(out), 256(in)]
                w2A[:, o, 0, :] = wt2[:, 0:128].T
                w2A[:, o, 1, :] = wt2[:, 128:256].T
                o += 1
    import ml_dtypes
    w1A = w1A.reshape(128, NOFF * 2 * 128).astype(ml_dtypes.bfloat16)
    w2A = w2A.reshape(128, NOFF * 2 * 128).astype(ml_dtypes.bfloat16)
    b1m = (sum(conv1_b) / 3.0).reshape(DFF, 1).astype(np.float32)
    b2m = (sum(conv2_b) / 3.0).astype(np.float32)
    ebv = (b2m @ enh_w + enh_b).reshape(D, 1).astype(np.float32)

    shared = dict(
        w1=w1A, w2=w2A, b1=b1m, eb=ebv,
        enh=np.ascontiguousarray(enh_w),
        ident=np.eye(128, dtype=np.float32),
        aw1=np.ascontiguousarray(np.asarray(attn_w1, dtype=np.float32).T
                                 ).reshape(1, K * 2 * K),
        aw2=np.ascontiguousarray(np.asarray(attn_w2, dtype=np.float32).T
                                 ).reshape(1, 2 * K * K),
        ab1=np.asarray(attn_b1, dtype=np.float32).reshape(1, 2 * K),
        ab2=np.asarray(attn_b2, dtype=np.float32).reshape(1, K),
    )
    in_maps = []
    for b in range(B):
        m = dict(shared)
        m["xb"] = np.ascontiguousarray(x[b])
        m["pw"] = pws[b]
        in_maps.append(m)
    return geoms, in_maps


def _ensure_axon_hooks():
    """bass_utils' trace path imports antenv.axon_hooks unconditionally;
    the container's antenv lacks it. Provide it, registering the real
    ctypes NTFF hook when available so tracing works."""
    try:
        import antenv.axon_hooks  # noqa: F401
        return
    except Exception:
        pass
    import sys
    import types

    import antenv

    m = types.ModuleType("antenv.axon_hooks")
    m._h = None
    m.set_axon_ntff_profile_hook = lambda h: setattr(m, "_h", h)
    m.get_axon_ntff_profile_hook = lambda: m._h
    sys.modules["antenv.axon_hooks"] = m
    antenv.axon_hooks = m
    try:
        from trn_agent_boot.trn_boot import _ntff_profile_via_ctypes

        m._h = _ntff_profile_via_ctypes("/opt/axon/libaxon_pjrt.so")
    except Exception:
        pass


def kernel(**inputs):
    global _last_exec_time_ns, _last_results
    _ensure_axon_hooks()
    geoms, in_maps = _prep_inputs(**inputs)
    nc = _build_program(geoms)
    res = run_bass_kernel_spmd(nc, in_maps, core_ids=list(range(8)))
    _last_results = res
    _last_exec_time_ns = res.exec_time_ns
    out = np.stack([res.results[i]["out"] for i in range(8)], axis=0)
    return out.astype(np.float32)


# revision 12
# speedup vs baseline: 1.3927x; 1.0194x over previous
"""AdaptivePeriodicLayer Trainium2 kernel.

Strategy: data-parallel over batch (8 samples -> 8 NeuronCores), one SPMD
program with an 8-way tc.Switch on partition id. Each arm is specialized at
build time to that sample's 3 FFT-detected period geometries (computed on
host, like the reference's .item() round-trip). Convs are computed as
per-kernel-offset matmuls accumulated in PSUM over a zero-padded 2D canvas
held in SBUF; conv operands are bf16 (fp32 PSUM accumulation), everything
else fp32.
"""

import numpy as np

import concourse.bacc as bacc
import concourse.bass as bass
import concourse.mybir as mybir
from concourse.tile import TileContext
from concourse.bass_utils import run_bass_kernel_spmd

F32 = mybir.dt.float32
BF16 = mybir.dt.bfloat16
AF = mybir.ActivationFunctionType
AX = mybir.AxisListType

B, S, D, DFF, K = 8, 512, 128, 256, 3
NOFF = 35  # 1 + 9 + 25 conv taps across the three inception branches
WCHUNK = 7  # offsets per weight DMA chunk
NWCH = 5    # 35 / 7

_last_exec_time_ns = None
_last_results = None


def _offsets():
    offs = []
    for ksz in (1, 3, 5):
        q = (ksz - 1) // 2
        for kh in range(ksz):
            for kw in range(ksz):
                offs.append((q, kh, kw))
    return offs


OFFS = _offsets()


def _row_chunks(p, n):
    """Split the p x n image (h-major) into row groups of <=512 columns."""
    ch = max(1, min(p, 512 // n))
    out = []
    h0 = 0
    while h0 < p:
        rows = min(ch, p - h0)
        out.append((h0, rows))
        h0 += rows
    return out


def _build_program(geoms):
    """geoms: [8][3] list of (p, n) per core per pass."""
    nc = bacc.Bacc()

    xb = nc.declare_dram_parameter("xb", [S, D], F32, isOutput=False)
    w1 = nc.declare_dram_parameter("w1", [D, NOFF * 2 * 128], BF16, isOutput=False)
    w2 = nc.declare_dram_parameter("w2", [D, NOFF * 2 * 128], BF16, isOutput=False)
    b1 = nc.declare_dram_parameter("b1", [DFF, 1], F32, isOutput=False)
    eb = nc.declare_dram_parameter("eb", [D, 1], F32, isOutput=False)
    enh = nc.declare_dram_parameter("enh", [D, D], BF16, isOutput=False)
    ident = nc.declare_dram_parameter("ident", [128, 128], F32, isOutput=False)
    aw1 = nc.declare_dram_parameter("aw1", [1, K * 2 * K], F32, isOutput=False)
    aw2 = nc.declare_dram_parameter("aw2", [1, 2 * K * K], F32, isOutput=False)
    ab1 = nc.declare_dram_parameter("ab1", [1, 2 * K], F32, isOutput=False)
    ab2 = nc.declare_dram_parameter("ab2", [1, K], F32, isOutput=False)
    pwd = nc.declare_dram_parameter("pw", [1, K], F32, isOutput=False)
    out_d = nc.declare_dram_parameter("out", [S, D], F32, isOutput=True)

    with TileContext(nc) as tc:
        with (
            tc.tile_pool(name="persist", bufs=1) as pp,
            tc.tile_pool(name="work", bufs=1) as wp,
            tc.tile_pool(name="psum", bufs=1, space="PSUM") as qp,
        ):
            # partition id + arm-prefetch hint as early as possible so the
            # IRAM block of this core's Switch arm is resident at dispatch
            pid = nc.partition_id(engines=mybir.ALL_ENGINES)
            shint = tc.switch_hint({e: pid for e in mybir.ALL_ENGINES}, 8,
                                   "arms")

            # ---- persistent loads (identical data on every core) ----
            identS = pp.tile([128, 128], F32, tag="identS", name="identS")
            nc.sync.dma_start(out=identS, in_=ident[:, :])

            xall = pp.tile([128, S], F32, tag="xall", name="xall")
            for c in range(4):
                nc.sync.dma_start(out=xall[:, c * 128:(c + 1) * 128],
                                  in_=xb[c * 128:(c + 1) * 128, :])

            w1c = []
            for ci in range(NWCH):
                t = pp.tile([128, WCHUNK * 2 * 128], BF16, tag=f"w1c{ci}",
                            name=f"w1c{ci}")
                nc.sync.dma_start(
                    out=t, in_=w1[:, ci * WCHUNK * 256:(ci + 1) * WCHUNK * 256])
                w1c.append(t)

            b1S = pp.tile([128, 2], F32, tag="b1S", name="b1S")
            nc.sync.dma_start(out=b1S[:, 0:1], in_=b1[0:128, :])
            nc.sync.dma_start(out=b1S[:, 1:2], in_=b1[128:256, :])
            ebS = pp.tile([128, 1], F32, tag="ebS", name="ebS")
            nc.sync.dma_start(out=ebS, in_=eb[:, :])
            enhS = pp.tile([128, 128], BF16, tag="enhS", name="enhS")
            nc.sync.dma_start(out=enhS, in_=enh[:, :])
            aw1S = pp.tile([1, K * 2 * K], F32, tag="aw1S", name="aw1S")
            nc.sync.dma_start(out=aw1S, in_=aw1[:, :])
            aw2S = pp.tile([1, 2 * K * K], F32, tag="aw2S", name="aw2S")
            nc.sync.dma_start(out=aw2S, in_=aw2[:, :])
            ab1S = pp.tile([1, 2 * K], F32, tag="ab1S", name="ab1S")
            nc.sync.dma_start(out=ab1S, in_=ab1[:, :])
            ab2S = pp.tile([1, K], F32, tag="ab2S", name="ab2S")
            nc.sync.dma_start(out=ab2S, in_=ab2[:, :])
            pwS = pp.tile([1, K], F32, tag="pwS", name="pwS")
            nc.sync.dma_start(out=pwS, in_=pwd[:, :])

            w2c = []
            for ci in range(NWCH):
                t = pp.tile([128, WCHUNK * 2 * 128], BF16, tag=f"w2c{ci}",
                            name=f"w2c{ci}")
                nc.sync.dma_start(
                    out=t, in_=w2[:, ci * WCHUNK * 256:(ci + 1) * WCHUNK * 256])
                w2c.append(t)

            # constants
            onesM = pp.tile([128, 1], F32, tag="onesM", name="onesM")
            nc.vector.memset(onesM, 1.0 / 128.0)
            ones1_128 = pp.tile([1, 128], F32, tag="ones1_128", name="ones1_128")
            nc.vector.memset(ones1_128, 1.0)

            # ---- transpose x: [S, D] -> xT [D, S] ----
            xT = pp.tile([128, S], F32, tag="xT", name="xT")
            for c in range(4):
                pt = qp.tile([128, 512], F32, tag="ps_small", bufs=2,
                             name=f"ptx{c}")
                nc.tensor.transpose(pt[:, 0:128],
                                    xall[:, c * 128:(c + 1) * 128], identS)
                nc.vector.tensor_copy(out=xT[:, c * 128:(c + 1) * 128],
                                      in_=pt[:, 0:128])

            def w1_lhsT(o, mc):
                ci, oi = divmod(o, WCHUNK)
                base = (oi * 2 + mc) * 128
                return w1c[ci][:, base:base + 128]

            def w2_lhsT(o, kc):
                ci, oi = divmod(o, WCHUNK)
                base = (oi * 2 + kc) * 128
                return w2c[ci][:, base:base + 128]

            def _transpose_po(core, k, po_k, poT_k):
                for c in range(4):
                    pt = qp.tile([128, 512], F32, tag="ps_small", bufs=2,
                                 name=f"ptp{k}{c}_{core}")
                    nc.tensor.transpose(pt[:, 0:128],
                                        po_k[:, c * 128:(c + 1) * 128],
                                        identS)
                    nc.vector.tensor_copy(
                        out=poT_k[:, c * 128:(c + 1) * 128], in_=pt[:, 0:128])

            def _fill_canvas(core, k, cv):
                p, n = geoms[core][k]
                nfull = S // p
                r = S - nfull * p
                src = xT[:, 0:nfull * p].rearrange("d (w h) -> d h w", h=p)
                nc.vector.tensor_copy(out=cv[:, 2:2 + p, 2:2 + nfull],
                                      in_=src)
                if r > 0:
                    tsrc = xT[:, nfull * p:S].rearrange("d (h w) -> d h w",
                                                        w=1)
                    nc.vector.tensor_copy(
                        out=cv[:, 2:2 + r, 2 + nfull:3 + nfull], in_=tsrc)

            def build_arm(core):
                # allocate per-pass tiles; memset canvases early so DVE works
                # while the weight DMAs stream in
                cvs, hcs = [], []
                for k in range(K):
                    p, n = geoms[core][k]
                    Hp, Wp = p + 4, n + 4
                    canvas = wp.tile([128, Hp * Wp], BF16, tag=f"canvas{k}",
                                     name=f"canvas{k}_{core}")
                    nc.gpsimd.memset(canvas, 0.0)
                    cvs.append(canvas.rearrange("d (h w) -> d h w", w=Wp))
                    if k == 0:
                        _fill_canvas(core, 0, cvs[0])
                    hk = []
                    for mc in range(2):
                        hc = wp.tile([128, Hp * Wp], BF16, tag=f"hc{k}_{mc}",
                                     name=f"hc{k}_{mc}_{core}")
                        nc.gpsimd.memset(hc, 0.0)
                        hk.append(hc.rearrange("d (h w) -> d h w", w=Wp))
                    hcs.append(hk)

                po, poT = [], []
                c2s, ps3s = {}, {}
                polastT = wp.tile([128, K], F32, tag="polast",
                                  name=f"polast_{core}")

                def finish_pass(k):
                    p, n = geoms[core][k]
                    L = p * n
                    nfull = S // p
                    r = S - nfull * p
                    c2 = c2s[k]
                    ps3 = qp.tile([128, 512], F32, tag="ps_small", bufs=2,
                                  name=f"ps3_{core}_{k}")
                    c2v = c2[:, 0:L].rearrange("d (h w) -> d w h", w=n)
                    nc.tensor.matmul(ps3[:, 0:nfull * p], enhS,
                                     c2v[:, 0:nfull, :],
                                     start=True, stop=(r == 0))
                    if r > 0:
                        nc.tensor.matmul(ps3[:, nfull * p:S], enhS,
                                         c2v[:, nfull:nfull + 1, 0:r],
                                         start=False, stop=True)
                    po_k = wp.tile([128, S], F32, tag=f"po{k}",
                                   name=f"po{k}_{core}")
                    nc.scalar.activation(out=po_k, in_=ps3[:, 0:S],
                                         func=AF.Gelu, bias=ebS[:, 0:1])
                    po.append(po_k)
                    nc.vector.tensor_copy(out=polastT[:, k:k + 1],
                                          in_=po_k[:, S - 1:S])
                    poT_k = wp.tile([128, S], F32, tag=f"poT{k}",
                                    name=f"poT{k}_{core}")
                    if k < 2:
                        _transpose_po(core, k, po_k, poT_k)
                    poT.append(poT_k)

                for k in range(K):
                    p, n = geoms[core][k]
                    L = p * n
                    nfull = S // p
                    r = S - nfull * p
                    chunks = _row_chunks(p, n)
                    assert len(chunks) <= 2, (p, n, chunks)
                    cv, hcv = cvs[k], hcs[k]

                    if k > 0:
                        _fill_canvas(core, k, cv)

                    # conv1: accumulate 35 taps per (Cout chunk, row chunk)
                    ps1 = [qp.tile([128, 1024], F32, tag=f"c1p{mc}",
                                   name=f"c1p{mc}_{core}_{k}")
                           for mc in range(2)]
                    for mc in range(2):
                        for o, (q, kh, kw) in enumerate(OFFS):
                            for j, (h0, rows) in enumerate(chunks):
                                rhs = cv[:, 2 - q + kh + h0:
                                         2 - q + kh + h0 + rows,
                                         2 - q + kw:2 - q + kw + n]
                                nc.tensor.matmul(
                                    ps1[mc][:, j * 512:j * 512 + rows * n],
                                    w1_lhsT(o, mc), rhs,
                                    start=(o == 0), stop=(o == NOFF - 1))
                        # gelu(conv1 + b1) written into padded h-canvas
                        for j, (h0, rows) in enumerate(chunks):
                            nc.scalar.activation(
                                out=hcv[mc][:, 2 + h0:2 + h0 + rows, 2:2 + n],
                                in_=ps1[mc][:, j * 512:j * 512 + rows * n],
                                func=AF.Gelu, bias=b1S[:, mc:mc + 1])

                    if k > 0:
                        finish_pass(k - 1)

                    # conv2: contract over 256 channels (2 K-chunks) x 35 taps
                    ps2 = qp.tile([128, 1024], F32, tag="c2p",
                                  name=f"c2p_{core}_{k}")
                    for kc in range(2):
                        for o, (q, kh, kw) in enumerate(OFFS):
                            for j, (h0, rows) in enumerate(chunks):
                                rhs = hcv[kc][:, 2 - q + kh + h0:
                                              2 - q + kh + h0 + rows,
                                              2 - q + kw:2 - q + kw + n]
                                nc.tensor.matmul(
                                    ps2[:, j * 512:j * 512 + rows * n],
                                    w2_lhsT(o, kc), rhs,
                                    start=(kc == 0 and o == 0),
                                    stop=(kc == 1 and o == NOFF - 1))

                    # conv2 out back to SBUF, contiguous h-major
                    c2 = wp.tile([128, 1024], BF16, tag="c2", bufs=2,
                                 name=f"c2_{core}_{k}")
                    for j, (h0, rows) in enumerate(chunks):
                        nc.vector.tensor_copy(
                            out=c2[:, h0 * n:h0 * n + rows * n],
                            in_=ps2[:, j * 512:j * 512 + rows * n])
                    c2s[k] = c2

                finish_pass(2)

                # ---- combination weights (free-dim chain on partition 0) ----
                lsfP = qp.tile([1, K], F32, tag="ps_small", bufs=2,
                               name=f"lsfP_{core}")
                nc.tensor.matmul(lsfP, onesM, polastT, start=True, stop=True)
                lsfS = wp.tile([1, K], F32, tag="lsfS", name=f"lsfS_{core}")
                nc.vector.tensor_copy(out=lsfS, in_=lsfP)

                # attn MLP on partition 0; aw1/aw2 stored column-major
                t6 = wp.tile([1, 2 * K], F32, tag="t6", name=f"t6_{core}")
                tt6 = wp.tile([1, 2 * K], F32, tag="tt6", name=f"tt6_{core}")
                aw1r = aw1S.rearrange("p (j i) -> p i j", i=K)
                nc.vector.tensor_scalar_mul(t6, aw1r[:, 0, :], lsfS[:, 0:1])
                nc.vector.tensor_scalar_mul(tt6, aw1r[:, 1, :], lsfS[:, 1:2])
                nc.vector.tensor_add(t6, t6, tt6)
                nc.vector.tensor_scalar_mul(tt6, aw1r[:, 2, :], lsfS[:, 2:3])
                nc.vector.tensor_add(t6, t6, tt6)
                nc.vector.tensor_add(t6, t6, ab1S)
                nc.vector.tensor_scalar_max(t6, t6, 0.0)

                a3 = wp.tile([1, K], F32, tag="a3", name=f"a3_{core}")
                tt3 = wp.tile([1, K], F32, tag="tt3", name=f"tt3_{core}")
                aw2r = aw2S.rearrange("p (j i) -> p i j", i=2 * K)
                nc.vector.tensor_scalar_mul(a3, aw2r[:, 0, :], t6[:, 0:1])
                for i in range(1, 6):
                    nc.vector.tensor_scalar_mul(tt3, aw2r[:, i, :],
                                                t6[:, i:i + 1])
                    nc.vector.tensor_add(a3, a3, tt3)
                nc.vector.tensor_add(a3, a3, ab2S)

                e1 = wp.tile([1, K], F32, tag="e1", name=f"e1_{core}")
                nc.scalar.activation(out=e1, in_=a3, func=AF.Exp)
                s1 = wp.tile([1, 1], F32, tag="s1", name=f"s1_{core}")
                nc.vector.reduce_sum(out=s1, in_=e1, axis=AX.X)
                r1 = wp.tile([1, 1], F32, tag="r1", name=f"r1_{core}")
                nc.vector.reciprocal(r1, s1)
                z = wp.tile([1, K], F32, tag="z", name=f"z_{core}")
                nc.vector.tensor_scalar_mul(z, e1, r1[:, 0:1])
                nc.vector.tensor_mul(z, z, pwS)
                e2 = wp.tile([1, K], F32, tag="e2", name=f"e2_{core}")
                nc.scalar.activation(out=e2, in_=z, func=AF.Exp)
                s2 = wp.tile([1, 1], F32, tag="s2", name=f"s2_{core}")
                nc.vector.reduce_sum(out=s2, in_=e2, axis=AX.X)
                r2 = wp.tile([1, 1], F32, tag="r2", name=f"r2_{core}")
                nc.vector.reciprocal(r2, s2)
                cwrow = wp.tile([1, K], F32, tag="cwrow", name=f"cwrow_{core}")
                nc.vector.tensor_scalar_mul(cwrow, e2, r2[:, 0:1])

                cwbP = qp.tile([128, K], F32, tag="ps_small", bufs=2,
                               name=f"cwbP_{core}")
                nc.tensor.matmul(cwbP, ones1_128, cwrow, start=True, stop=True)
                cwS = wp.tile([128, K], F32, tag="cwS", name=f"cwS_{core}")
                nc.vector.tensor_copy(out=cwS, in_=cwbP)

                _transpose_po(core, 2, po[2], poT[2])
                # ---- weighted combine + residual, already in [s, d] ----
                accT = wp.tile([128, S], F32, tag="accT", name=f"accT_{core}")
                q1T = wp.tile([128, S], F32, tag="q1T", name=f"q1T_{core}")
                q2T = wp.tile([128, S], F32, tag="q2T", name=f"q2T_{core}")
                nc.scalar.activation(out=q1T, in_=poT[1], func=AF.Copy,
                                     scale=cwS[:, 1:2])
                nc.scalar.activation(out=q2T, in_=poT[2], func=AF.Copy,
                                     scale=cwS[:, 2:3])
                nc.vector.tensor_scalar_mul(accT, poT[0], cwS[:, 0:1])
                nc.vector.tensor_add(accT, accT, xall)
                nc.vector.tensor_add(accT, accT, q1T)
                nc.vector.tensor_add(accT, accT, q2T)

                nc.sync.dma_start(
                    out=out_d.rearrange("(c sp) d -> sp c d", sp=128),
                    in_=accT.rearrange("sp (c d) -> sp c d", d=128))

            for core in tc.Switch(pid, 8, hint=shint):
                build_arm(core)

    nc.finalize()
    return nc


def _prep_inputs(x, conv1_w, conv1_b, conv2_w, conv2_b, attn_w1, attn_b1,
                 attn_w2, attn_b2, enh_w, enh_b):
    x = np.asarray(x, dtype=np.float32)
    conv1_w = [np.asarray(w, dtype=np.float32) for w in conv1_w]
    conv1_b = [np.asarray(w, dtype=np.float32) for w in conv1_b]
    conv2_w = [np.asarray(w, dtype=np.float32) for w in conv2_w]
    conv2_b = [np.asarray(w, dtype=np.float32) for w in conv2_b]
    enh_w = np.asarray(enh_w, dtype=np.float32)
    enh_b = np.asarray(enh_b, dtype=np.float32)

    # FFT period detection on host (reference does the same via .item())
    xf = np.abs(np.fft.rfft(x, axis=1)).mean(-1).astype(np.float32)
    xf[:, 0] = 0.0
    top = np.argsort(-xf, axis=1, kind="stable")[:, :K]
    geoms, pws = [], []
    for b in range(B):
        g = []
        for k in range(K):
            f = int(top[b, k])
            p = S if f == 0 else max(1, S // f)
            pad = (-S) % p
            g.append((p, (S + pad) // p))
        geoms.append(g)
        wv = xf[b, top[b]].astype(np.float64)
        ev = np.exp(wv - wv.max())
        pws.append((ev / ev.sum()).astype(np.float32).reshape(1, K))

    # conv weights -> per-tap transposed matmul operands, pre-scaled by 1/3
    w1A = np.zeros((128, NOFF, 2, 128), dtype=np.float32)
    w2A = np.zeros((128, NOFF, 2, 128), dtype=np.float32)
    o = 0
    for br, ksz in enumerate((1, 3, 5)):
        for kh in range(ksz):
            for kw in range(ksz):
                wt1 = conv1_w[br][:, :, kh, kw] / 3.0   # [256(out), 128(in)]
                w1A[:, o, 0, :] = wt1[0:128, :].T
                w1A[:, o, 1, :] = wt1[128:256, :].T
                wt2 = conv2_w[br][:, :, kh, kw] / 3.0   # [128(out), 256(in)]
                w2A[:, o, 0, :] = wt2[:, 0:128].T
                w2A[:, o, 1, :] = wt2[:, 128:256].T
                o += 1
    import ml_dtypes
    w1A = w1A.reshape(128, NOFF * 2 * 128).astype(ml_dtypes.bfloat16)
    w2A = w2A.reshape(128, NOFF * 2 * 128).astype(ml_dtypes.bfloat16)
    b1m = (sum(conv1_b) / 3.0).reshape(DFF, 1).astype(np.float32)
    b2m = (sum(conv2_b) / 3.0).astype(np.float32)
    ebv = (b2m @ enh_w + enh_b).reshape(D, 1).astype(np.float32)

    shared = dict(
        w1=w1A, w2=w2A, b1=b1m, eb=ebv,
        enh=np.ascontiguousarray(enh_w).astype(ml_dtypes.bfloat16),
        ident=np.eye(128, dtype=np.float32),
        aw1=np.ascontiguousarray(np.asarray(attn_w1, dtype=np.float32).T
                                 ).reshape(1, K * 2 * K),
        aw2=np.ascontiguousarray(np.asarray(attn_w2, dtype=np.float32).T
                                 ).reshape(1, 2 * K * K),
        ab1=np.asarray(attn_b1, dtype=np.float32).reshape(1, 2 * K),
        ab2=np.asarray(attn_b2, dtype=np.float32).reshape(1, K),
    )
    in_maps = []
    for b in range(B):
        m = dict(shared)
        m["xb"] = np.ascontiguousarray(x[b])
        m["pw"] = pws[b]
        in_maps.append(m)
    return geoms, in_maps


def _ensure_axon_hooks():
    """bass_utils' trace path imports antenv.axon_hooks unconditionally;
    the container's antenv lacks it. Provide it, registering the real
    ctypes NTFF hook when available so tracing works."""
    try:
        import antenv.axon_hooks  # noqa: F401
        return
    except Exception:
        pass
    import sys
    import types

    import antenv

    m = types.ModuleType("antenv.axon_hooks")
    m._h = None
    m.set_axon_ntff_profile_hook = lambda h: setattr(m, "_h", h)
    m.get_axon_ntff_profile_hook = lambda: m._h
    sys.modules["antenv.axon_hooks"] = m
    antenv.axon_hooks = m
    try:
        from trn_agent_boot.trn_boot import _ntff_profile_via_ctypes

        m._h = _ntff_profile_via_ctypes("/opt/axon/libaxon_pjrt.so")
    except Exception:
        pass


def kernel(**inputs):
    global _last_exec_time_ns, _last_results
    _ensure_axon_hooks()
    geoms, in_maps = _prep_inputs(**inputs)
    nc = _build_program(geoms)
    res = run_bass_kernel_spmd(nc, in_maps, core_ids=list(range(8)))
    _last_results = res
    _last_exec_time_ns = res.exec_time_ns
    out = np.stack([res.results[i]["out"] for i in range(8)], axis=0)
    return out.astype(np.float32)
